# revision 1
# baseline (speedup 1.0000x reference)
"""QLoRA-style MLP (fake-quant base + fp32 LoRA + exact GeLU) on 8 TRN2 cores.

Sharding: token data-parallel (4096 tokens / 8 cores = 512 tokens per core),
weights replicated.  The only cross-core communication is a tiny AllReduce(max)
for the global fake-quant scale of the hidden activation.

Math per layer (matching the jax reference):
    base = fq(x) @ fq(W) + b          fq(t) = clip(round(t/s), -127, 127) * s,
                                      s = max(max|t|, 1e-8) / 127  (global max)
    lora = 2.0 * (x @ A) @ B          (full precision)
    out  = base + lora                (layer 1 additionally GeLU'd, exact erf)

Device mapping (per core, T=512 tokens):
  L1: psum[ff128, T] = sum_k qW_fc[k,ff]^T-tiles @ qxT[k,T]   (bf16 exact ints)
                       + B_fc[16,ff]^T @ ((x@A_fc)^T * 2/s1)  (fp32, K=16)
      hT = Gelu(psum * s1 + b_fc)  -> abs-max tracked, spilled fp32 to HBM
  AllReduce(max) -> s_h;  qhT = round(hT/s_h) as bf16 (magic-number rounding)
  L2: psum[tok128, d] = sum_k qhT[k,tok]-tiles @ qW_proj[k,d]
                        + ((h@A_proj)^T * 2/s2) @ B_proj      (fp32, K=16)
      out = psum * s2 + b_proj
"""

import os
import sys

import numpy as np

if "/opt/trn_rl_repo" not in sys.path:
    sys.path.insert(0, "/opt/trn_rl_repo")

import ml_dtypes

# Problem shapes (hardcoded per contract).
B_, S, D, DFF, R = 2, 2048, 2048, 8192, 16
T = B_ * S  # 4096 tokens
NCORES = 8
TC = T // NCORES  # 512 tokens per core
QMAX = np.float32(127.0)
MAGIC = float(np.float32(12582912.0))  # 1.5 * 2**23: fp32 round-half-even trick

KO1 = D // 128  # 16  k-tiles for layer 1
MO1 = DFF // 512  # 16  512-wide ff blocks
M64 = DFF // 128  # 64  128-wide ff blocks
KO2 = DFF // 128  # 64  k-tiles for layer 2
NO2 = D // 512  # 4   512-wide output-col blocks
MT = TC // 128  # 4   token tiles per core

_CACHE = {}
LAST_RESULT = None  # test harness can read exec_time_ns etc. from here


def _build_nc(n_cores=NCORES, tc_=TC, d_=D, dff_=DFF, dmodel_=D, act="gelu", flags=()):
    """Build + compile the Bass program. Dimensions parameterizable for sim tests."""
    from contextlib import ExitStack

    import concourse.bass as bass  # noqa: F401
    import concourse.mybir as mybir
    import concourse.tile as tile
    from concourse import bacc, bass_isa
    from concourse.bass import ds, ts

    f32 = mybir.dt.float32
    bf16 = mybir.dt.bfloat16
    AF = mybir.ActivationFunctionType
    ALU = mybir.AluOpType

    ko1 = d_ // 128
    mo1 = dff_ // 512
    m64 = dff_ // 128
    ko2 = dff_ // 128
    no2 = dmodel_ // 512
    mt = tc_ // 128

    nc = bacc.Bacc(None, target_bir_lowering=False, num_devices=n_cores)

    # ---- kernel I/O -------------------------------------------------------
    qx_t = nc.dram_tensor("qx_t", [128, ko1, tc_], bf16, kind="ExternalInput")
    xt_t = nc.dram_tensor("xt_t", [128, ko1, tc_], f32, kind="ExternalInput")
    wfc_t = nc.dram_tensor("wfc_t", [mo1, ko1, 128, 512], bf16, kind="ExternalInput")
    afc_t = nc.dram_tensor("afc_t", [128, ko1, R], f32, kind="ExternalInput")
    bfcl_t = nc.dram_tensor("bfcl_t", [R, dff_], f32, kind="ExternalInput")
    biasfc_t = nc.dram_tensor("biasfc_t", [128, m64], f32, kind="ExternalInput")
    wproj_t = nc.dram_tensor("wproj_t", [ko2, 128, no2, 512], bf16, kind="ExternalInput")
    aproj_t = nc.dram_tensor("aproj_t", [128, ko2, R], f32, kind="ExternalInput")
    bprojl_t = nc.dram_tensor("bprojl_t", [R, dmodel_], f32, kind="ExternalInput")
    biasproj_t = nc.dram_tensor("biasproj_t", [128, dmodel_], f32, kind="ExternalInput")
    # scal columns: 0: s1 = s_x*s_wfc, 1: c1 = 2/s1, 2: s_wproj  (host replicates x128)
    scal_t = nc.dram_tensor("scal_t", [128, 4], f32, kind="ExternalInput")
    out_t = nc.dram_tensor("out", [mt, 128, dmodel_], f32, kind="ExternalOutput")

    with tile.TileContext(nc) as tc:
        with ExitStack() as ctx:
            consts = ctx.enter_context(tc.tile_pool(name="consts", bufs=1))
            dram = ctx.enter_context(tc.tile_pool(name="dram", bufs=1, space="DRAM"))
            lora_on = "no_lora" not in flags

            # whole-kernel residents
            scal_sb = consts.tile([128, 4], f32)
            qh_sb = consts.tile([128, ko2, tc_], bf16)
            xa2_sb = consts.tile([R, tc_], f32)
            h_dram = dram.tile([m64, 128, tc_], f32)
            ar_in = dram.tile([128, 1], f32)
            ar_out = dram.tile(
                [128, 1], f32, addr_space="Shared" if n_cores > 4 else "Local"
            )
            nc.sync.dma_start(scal_sb[:], scal_t[:])
            maxcol = consts.tile([128, m64], f32)
            aproj_sb = consts.tile([128, ko2, R], f32)
            if lora_on:
                nc.scalar.dma_start(aproj_sb[:], aproj_t[:])

            # ---- phase 1: hT = Gelu(s1 * (qx@qW + lora1/s1) + b_fc) ----------
            # ko-outer / sub-inner with per-ko weight tiles; qx and xt stream in
            # per-ko chunks so the PE starts ~2us in.  mo=0's lora add-ins are
            # deferred until the x@A_fc prologue (emitted mid-stream) finishes.
            with tc.tile_pool(name="ph1c", bufs=1) as ph1c, tc.tile_pool(
                name="wfc", bufs=8
            ) as wp, tc.tile_pool(name="hb1", bufs=6) as hp, tc.tile_pool(
                name="ps1", bufs=2, space="PSUM"
            ) as pp:
                qx_sb = ph1c.tile([128, ko1, tc_], bf16)
                afc_sb = ph1c.tile([128, ko1, R], f32)
                bfcl_sb = ph1c.tile([R, dff_], f32)
                biasfc_sb = ph1c.tile([128, m64], f32)
                xa1_sb = ph1c.tile([R, tc_], f32)
                xt_sb = ph1c.tile([128, ko1, tc_], f32)
                if lora_on:
                    nc.scalar.dma_start(afc_sb[:], afc_t[:])
                    nc.scalar.dma_start(bfcl_sb[:], bfcl_t[:])
                    nc.scalar.dma_start(biasfc_sb[:], biasfc_t[:])

                def l1_epilogue(ps, mi):
                    h_sb = hp.tile([128, tc_], f32, tag="h", name="h_sb")
                    nc.scalar.activation(
                        h_sb[:],
                        ps[:],
                        AF.Gelu if act == "gelu" else AF.Tanh,
                        bias=biasfc_sb[:, mi : mi + 1],
                        scale=scal_sb[:, 0:1],
                    )
                    nc.vector.tensor_reduce(
                        maxcol[:, mi : mi + 1],
                        h_sb[:],
                        axis=mybir.AxisListType.X,
                        op=ALU.max,
                        apply_absolute_value=True,
                    )
                    nc.scalar.dma_start(h_dram[mi], h_sb[:])

                for mo in range(mo1):
                    pss = [
                        pp.tile([128, tc_], f32, tag=f"ps1_{i}", name="ps1t")
                        for i in range(4)
                    ]
                    for ko in range(ko1):
                        if mo == 0:
                            nc.sync.dma_start(qx_sb[:, ko, :], qx_t[:, ko, :])
                        w_ko = wp.tile([128, 512], bf16, tag="wfc", name="w_ko")
                        nc.sync.dma_start(w_ko[:], wfc_t[mo, ko])
                        if mo == 0 and lora_on:
                            nc.sync.dma_start(xt_sb[:, ko, :], xt_t[:, ko, :])
                        for sub in range(4):
                            nc.tensor.matmul(
                                pss[sub][:],
                                w_ko[:, ts(sub, 128)],
                                qx_sb[:, ko, :],
                                start=(ko == 0),
                                stop=(not lora_on and ko == ko1 - 1),
                            )
                    if mo == 0 and lora_on:
                        # xa1 = (x @ A_fc)^T * (2/s1); xt chunks already in flight
                        ps_a1 = pp.tile([128, tc_], f32, tag="ps1_0", name="psa1")
                        for ko in range(ko1):
                            nc.tensor.matmul(
                                ps_a1[:R, :],
                                afc_sb[:, ko, :],
                                xt_sb[:, ko, :],
                                start=(ko == 0),
                                stop=(ko == ko1 - 1),
                            )
                        nc.vector.tensor_scalar_mul(
                            xa1_sb[:], ps_a1[:R, :], scal_sb[:R, 1:2]
                        )
                    elif mo == 0:
                        nc.sync.dma_start(biasfc_sb[:], biasfc_t[:])
                    for sub in range(4):
                        mi = 4 * mo + sub
                        if lora_on:
                            nc.tensor.matmul(
                                pss[sub][:],
                                bfcl_sb[:, ds(mi * 128, 128)],
                                xa1_sb[:],
                                start=False,
                                stop=True,
                            )
                        l1_epilogue(pss[sub], mi)

            # ---- phase 1.5: global scale via AllReduce(max) ------------------
            pmax = consts.tile([128, 1], f32)
            nc.vector.tensor_reduce(
                pmax[:], maxcol[:], axis=mybir.AxisListType.X, op=ALU.max
            )
            armax = consts.tile([128, 1], f32)
            if "no_collective" in flags:
                nc.vector.tensor_copy(armax[:], pmax[:])
            else:
                nc.gpsimd.dma_start(ar_in[:], pmax[:])
                nc.gpsimd.collective_compute(
                    "AllReduce",
                    ALU.max,
                    replica_groups=[list(range(n_cores))],
                    ins=[ar_in[:]],
                    outs=[ar_out[:]],
                )
                nc.gpsimd.dma_start(armax[:], ar_out[:])
            gmax = consts.tile([128, 1], f32)
            if "no_par_reduce" in flags:
                nc.vector.tensor_copy(gmax[:], armax[:])
            else:
                nc.gpsimd.partition_all_reduce(
                    gmax[:], armax[:], channels=128, reduce_op=bass_isa.ReduceOp.max
                )
            scaleh = consts.tile([128, 1], f32)
            invsh = consts.tile([128, 1], f32)
            s2v = consts.tile([128, 1], f32)
            c2v = consts.tile([128, 1], f32)
            nc.vector.tensor_scalar_max(gmax[:], gmax[:], 1e-8)
            # scale_h = gmax / 127  (multiply by fp32(1/127): <=1ulp from divide)
            nc.vector.tensor_scalar_mul(
                scaleh[:], gmax[:], float(np.float32(1.0) / np.float32(127.0))
            )
            nc.vector.reciprocal(invsh[:], scaleh[:])
            nc.vector.tensor_tensor(s2v[:], scaleh[:], scal_sb[:, 2:3], op=ALU.mult)
            nc.vector.reciprocal(c2v[:], s2v[:])
            nc.vector.tensor_scalar_mul(c2v[:], c2v[:], 2.0)

            # ---- phase 2: out = s2 * (qh@qW2 + lora2/s2) + b_proj ------------
            # pre-loop interleaves hb reads with the no=0 w2 stream on the sync
            # queue; the PE's 64 s_h-independent xa2 matmuls cross the
            # AllReduce barrier while qh production (ACT*DVE) waits for s_h.
            with tc.tile_pool(name="ph2c", bufs=1) as ph2c, tc.tile_pool(
                name="w2", bufs=32
            ) as w2p, tc.tile_pool(name="hback", bufs=12) as hbp, tc.tile_pool(
                name="qt", bufs=3
            ) as qtp, tc.tile_pool(name="ps2", bufs=2, space="PSUM") as pp2, tc.tile_pool(
                name="ot", bufs=4
            ) as otp:
                bprojl_sb = ph2c.tile([R, dmodel_], f32)
                biasproj_sb = ph2c.tile([128, dmodel_], f32)
                if lora_on:
                    nc.scalar.dma_start(bprojl_sb[:], bprojl_t[:])
                nc.scalar.dma_start(biasproj_sb[:], biasproj_t[:])

                ps_a2 = pp2.tile([128, tc_], f32, tag="ps2_0", name="psa2")
                w2_saved = []
                for ko in range(ko2):
                    hb = hbp.tile([128, tc_], f32, tag="hb", name="hb")
                    nc.sync.dma_start(hb[:], h_dram[ko])
                    w2_sb = w2p.tile([128, 512], bf16, tag="w2", name="w2_sb")
                    nc.sync.dma_start(w2_sb[:], wproj_t[ko, :, 0, :])
                    w2_saved.append(w2_sb)
                    if lora_on:
                        nc.tensor.matmul(
                            ps_a2[:R, :],
                            aproj_sb[:, ko, :],
                            hb[:],
                            start=(ko == 0),
                            stop=(ko == ko2 - 1),
                        )
                    qt = qtp.tile([128, tc_], f32, tag="qt", name="qt")
                    if "no_act_ap" in flags:
                        nc.vector.tensor_tensor(
                            qt[:],
                            hb[:],
                            invsh[:, 0:1].to_broadcast((128, tc_)),
                            op=ALU.mult,
                        )
                    else:
                        nc.scalar.activation(
                            qt[:], hb[:], AF.Copy, bias=0.0, scale=invsh[:, 0:1]
                        )
                    nc.vector.tensor_scalar(
                        qh_sb[:, ko, :],
                        qt[:],
                        MAGIC,
                        MAGIC,
                        op0=ALU.add,
                        op1=ALU.subtract,
                    )
                if lora_on:
                    nc.vector.tensor_scalar_mul(xa2_sb[:], ps_a2[:R, :], c2v[:R, 0:1])

                for no in range(no2):
                    ps_list = []
                    for mi in range(mt):
                        ps2 = pp2.tile([128, 512], f32, tag=f"ps2_{mi}", name="ps2t")
                        ps_list.append(ps2)
                    for ko in range(ko2):
                        if no == 0:
                            w2_sb = w2_saved[ko]
                        else:
                            w2_sb = w2p.tile([128, 512], bf16, tag="w2", name="w2_sb")
                            nc.sync.dma_start(w2_sb[:], wproj_t[ko, :, no, :])
                        for mi in range(mt):
                            nc.tensor.matmul(
                                ps_list[mi][:],
                                qh_sb[:, ko, ts(mi, 128)],
                                w2_sb[:],
                                start=(ko == 0),
                                stop=(not lora_on and ko == ko2 - 1),
                            )
                    for mi in range(mt):
                        if lora_on:
                            nc.tensor.matmul(
                                ps_list[mi][:],
                                xa2_sb[:, ts(mi, 128)],
                                bprojl_sb[:, ds(no * 512, 512)],
                                start=False,
                                stop=True,
                            )
                        ot = otp.tile([128, 512], f32, tag="ot", name="ot")
                        # scale on ACT, bias-add on DVE (halves eviction latency
                        # at psum-bank reuse boundaries)
                        nc.scalar.activation(
                            ot[:], ps_list[mi][:], AF.Copy, bias=0.0, scale=s2v[:, 0:1]
                        )
                        nc.vector.tensor_add(
                            ot[:], ot[:], biasproj_sb[:, ds(no * 512, 512)]
                        )
                        nc.scalar.dma_start(out_t[mi, :, ds(no * 512, 512)], ot[:])

    nc.compile()
    return nc


def _scale_of(a):
    m = np.max(np.abs(a)).astype(np.float32)
    m = np.maximum(m, np.float32(1e-8))
    return (m / QMAX).astype(np.float32)


def _quant(a, s):
    return np.clip(np.round(a / s), -QMAX, QMAX)


def _prep_inputs(hidden_states, W_fc, b_fc, A_fc, B_fc, W_proj, b_proj, A_proj, B_proj):
    bf16 = ml_dtypes.bfloat16
    x = np.ascontiguousarray(np.asarray(hidden_states, np.float32).reshape(T, D))
    W_fc = np.asarray(W_fc, np.float32)
    W_proj = np.asarray(W_proj, np.float32)

    s_x = _scale_of(x)
    s_wfc = _scale_of(W_fc)
    s_wp = _scale_of(W_proj)
    qx = _quant(x, s_x)  # fp32 integer-valued
    qwfc = _quant(W_fc, s_wfc)
    qwp = _quant(W_proj, s_wp)

    s1 = np.float32(s_x * s_wfc)
    c1 = np.float32(np.float32(2.0) / s1)
    scal_row = np.array([s1, c1, s_wp, 0.0], np.float32)
    scal = np.ascontiguousarray(np.tile(scal_row, (128, 1)))

    wfc_dev = np.ascontiguousarray(
        qwfc.reshape(KO1, 128, MO1, 512).transpose(2, 0, 1, 3).astype(bf16)
    )
    wproj_dev = np.ascontiguousarray(qwp.reshape(KO2, 128, NO2, 512).astype(bf16))
    afc_dev = np.ascontiguousarray(
        np.asarray(A_fc, np.float32).reshape(KO1, 128, R).transpose(1, 0, 2)
    )
    aproj_dev = np.ascontiguousarray(
        np.asarray(A_proj, np.float32).reshape(KO2, 128, R).transpose(1, 0, 2)
    )
    bfcl_dev = np.ascontiguousarray(np.asarray(B_fc, np.float32))
    bprojl_dev = np.ascontiguousarray(np.asarray(B_proj, np.float32))
    biasfc_dev = np.ascontiguousarray(np.asarray(b_fc, np.float32).reshape(M64, 128).T)
    biasproj_dev = np.ascontiguousarray(
        np.tile(np.asarray(b_proj, np.float32)[None, :], (128, 1))
    )

    shared = {
        "wfc_t": wfc_dev,
        "afc_t": afc_dev,
        "bfcl_t": bfcl_dev,
        "biasfc_t": biasfc_dev,
        "wproj_t": wproj_dev,
        "aproj_t": aproj_dev,
        "bprojl_t": bprojl_dev,
        "biasproj_t": biasproj_dev,
        "scal_t": scal,
    }
    in_maps = []
    for c in range(NCORES):
        xc = x[c * TC : (c + 1) * TC]  # [TC, D]
        qxc = qx[c * TC : (c + 1) * TC]
        qxT = np.ascontiguousarray(
            qxc.T.reshape(KO1, 128, TC).transpose(1, 0, 2).astype(bf16)
        )
        xT = np.ascontiguousarray(xc.T.reshape(KO1, 128, TC).transpose(1, 0, 2))
        in_maps.append({**shared, "qx_t": qxT, "xt_t": xT})
    return in_maps


def _get_runner(**build_kwargs):
    """Build the Bass program once and wrap it in a cached jitted shard_map
    executable (adapted from concourse.bass2jax.run_bass_via_pjrt)."""
    key = ("runner", tuple(sorted(build_kwargs.items())))
    if key in _CACHE:
        return _CACHE[key]

    import jax
    import jax.numpy as jnp  # noqa: F401
    from jax.experimental.shard_map import shard_map
    from jax.sharding import Mesh, PartitionSpec

    from concourse import bass2jax, mybir

    nc = _build_nc(**build_kwargs)
    n_cores_ = build_kwargs.get("n_cores", NCORES)
    bass2jax.install_neuronx_cc_hook()
    assert nc.dbg_addr is None
    partition_name = nc.partition_id_tensor.name if nc.partition_id_tensor else None

    in_names = []
    out_names = []
    out_avals = []
    for alloc in nc.m.functions[0].allocations:
        if not isinstance(alloc, mybir.MemoryLocationSet):
            continue
        name = alloc.memorylocations[0].name
        if alloc.kind == "ExternalInput":
            if name != partition_name:
                in_names.append(name)
        elif alloc.kind == "ExternalOutput":
            out_names.append(name)
            out_avals.append(
                jax.core.ShapedArray(tuple(alloc.tensor_shape), mybir.dt.np(alloc.dtype))
            )
    all_in_names = tuple(in_names) + tuple(out_names)
    if partition_name is not None:
        all_in_names = all_in_names + (partition_name,)
    n_params = len(in_names)
    n_outs = len(out_names)

    def _body(*args):
        operands = list(args)
        if partition_name is not None:
            operands.append(bass2jax.partition_id_tensor())
        outs = bass2jax._bass_exec_p.bind(
            *operands,
            out_avals=tuple(out_avals),
            in_names=all_in_names,
            out_names=tuple(out_names),
            lowering_input_output_aliases=(),
            sim_require_finite=True,
            sim_require_nnan=True,
            nc=nc,
        )
        return tuple(outs)

    devices = jax.devices()[:n_cores_]
    assert len(devices) == n_cores_, f"need {n_cores_} devices, have {len(jax.devices())}"
    mesh = Mesh(np.asarray(devices), ("core",))
    in_specs = (PartitionSpec("core"),) * (n_params + n_outs)
    out_specs = (PartitionSpec("core"),) * n_outs
    donate = tuple(range(n_params, n_params + n_outs))
    fn = jax.jit(
        shard_map(
            _body, mesh=mesh, in_specs=in_specs, out_specs=out_specs, check_rep=False
        ),
        donate_argnums=donate,
        keep_unused=True,
    )
    runner = {
        "fn": fn,
        "in_names": in_names,
        "out_names": out_names,
        "out_avals": out_avals,
        "mesh": mesh,
    }
    runner["n_cores"] = n_cores_
    _CACHE[key] = runner
    return runner


def _zero_outs(runner):
    n = runner["n_cores"]
    return [
        np.zeros((n * a.shape[0], *a.shape[1:]), a.dtype) for a in runner["out_avals"]
    ]


def _concat_inputs(in_maps, in_names):
    return [
        np.concatenate([m[name] for m in in_maps], axis=0) for name in in_names
    ]


def kernel(hidden_states, W_fc, b_fc, A_fc, B_fc, W_proj, b_proj, A_proj, B_proj):
    global LAST_RESULT
    runner = _get_runner()
    in_maps = _prep_inputs(
        hidden_states, W_fc, b_fc, A_fc, B_fc, W_proj, b_proj, A_proj, B_proj
    )
    concat_in = _concat_inputs(in_maps, runner["in_names"])
    out_arrs = runner["fn"](*concat_in, *_zero_outs(runner))
    (out_global,) = [np.asarray(a) for a in out_arrs]
    # out_global: [NCORES*MT, 128, D] -> per-core [MT,128,D] -> tokens x D
    out = out_global.reshape(T, D).astype(np.float32)
    return out.reshape(B_, S, D)


def bench(n_iters=20, in_maps=None):
    """Steady-state per-iteration wall time of the compiled executable with
    device-resident inputs (upper bound on HW exec time; includes dispatch)."""
    import time

    import jax

    runner = _get_runner()
    if in_maps is None:
        rng = np.random.default_rng(0)
        dummy = {
            "hidden_states": rng.standard_normal((B_, S, D), dtype=np.float32),
            "W_fc": rng.standard_normal((D, DFF), dtype=np.float32) / 45.0,
            "b_fc": np.zeros(DFF, np.float32),
            "A_fc": rng.standard_normal((D, R), dtype=np.float32) / 45.0,
            "B_fc": rng.standard_normal((R, DFF), dtype=np.float32) * 0.01,
            "W_proj": rng.standard_normal((DFF, D), dtype=np.float32) / 90.0,
            "b_proj": np.zeros(D, np.float32),
            "A_proj": rng.standard_normal((DFF, R), dtype=np.float32) / 90.0,
            "B_proj": rng.standard_normal((R, D), dtype=np.float32) * 0.01,
        }
        in_maps = _prep_inputs(**dummy)
    concat_in = _concat_inputs(in_maps, runner["in_names"])
    from jax.sharding import NamedSharding, PartitionSpec

    sharding = NamedSharding(runner["mesh"], PartitionSpec("core"))
    dev_in = [jax.device_put(a, sharding) for a in concat_in]
    # donated output buffers are consumed per call: pre-stage one set per iter
    zero_sets = [
        [jax.device_put(z, sharding) for z in _zero_outs(runner)]
        for _ in range(n_iters + 1)
    ]
    out = runner["fn"](*dev_in, *zero_sets[-1])
    jax.block_until_ready(out)
    t0 = time.time()
    for i in range(n_iters):
        out = runner["fn"](*dev_in, *zero_sets[i])
    jax.block_until_ready(out)
    dt = (time.time() - t0) / n_iters
    return dt



# revision 5
# speedup vs baseline: 1.3936x; 1.3936x over previous
"""QLoRA-style MLP (fake-quant base + LoRA + exact GeLU) on 8 TRN2 cores.

Sharding: token data-parallel (4096 tokens / 8 cores = 512 tokens per core),
weights replicated.  The only cross-core communication is a tiny AllReduce(max)
for the global fake-quant scale of the hidden activation.

Math per layer (matching the jax reference):
    base = fq(x) @ fq(W) + b          fq(t) = clip(round(t/s), -127, 127) * s,
                                      s = max(max|t|, 1e-8) / 127  (global max)
    lora = 2.0 * (x @ A) @ B          (bf16 operands on device)
    out  = base + lora                (layer 1 additionally GeLU'd, exact erf)

v3 design (vs v1): h kept resident in SBUF as fp16 (no HBM spill/reload),
all LoRA matmuls stream bf16 (1 cycle/row on the PE instead of 4 for f32),
inputs packed into 6 DRAM tensors, weights repacked host-side so streaming
DMAs are few and large (16 x 2MB for W_fc, 32 x 1MB for W_proj).

Device mapping (per core, T=512 tokens):
  L1: psum[ff128, T] = sum_ko wfc[mo][k,ff]^T @ qx[k,T]      (bf16 int matmul)
                       + B_fc[16,ff]^T-slice @ xa1[16,T]     (bf16, K=16)
      h[ff,T](fp16) = Gelu(psum * s1 + b_fc); track per-column absmax
  AllReduce(max) -> s_h
  L2: xa2[16,T] = A_proj^T @ h  (64 bf16 matmuls, overlaps the AllReduce)
      qh[ff,T](bf16) = round(h / s_h)   (ACT scale + DVE magic-round)
      psum[tok128, 512] = sum_ko qh[k,tok]^T-tiles @ wproj[no][k,512]
                          + xa2[16,tok]^T-slice @ B_proj[16,512]
      out = psum * s2 + b_proj
"""

import os
import sys

import numpy as np

if "/opt/trn_rl_repo" not in sys.path:
    sys.path.insert(0, "/opt/trn_rl_repo")

import ml_dtypes

# Problem shapes (hardcoded per contract).
B_, S, D, DFF, R = 2, 2048, 2048, 8192, 16
T = B_ * S  # 4096 tokens
NCORES = 8
TC = T // NCORES  # 512 tokens per core
QMAX = np.float32(127.0)
MAGIC = float(np.float32(12582912.0))  # 1.5 * 2**23: fp32 round-half-even trick

KO1 = D // 128  # 16  k-tiles for layer 1
MO1 = DFF // 512  # 16  512-wide ff blocks
M64 = DFF // 128  # 64  128-wide ff blocks
KO2 = DFF // 128  # 64  k-tiles for layer 2
NO2 = D // 512  # 4   512-wide output-col blocks
MT = TC // 128  # 4   token tiles per core
W2CH = 8  # W_proj ko-tiles per streamed chunk
NCH2 = KO2 // W2CH  # 8 chunks per no

_CACHE = {}
LAST_RESULT = None


def _build_nc(n_cores=NCORES, flags=()):
    """Build + compile the Bass program."""
    from contextlib import ExitStack

    import concourse.bass as bass  # noqa: F401
    import concourse.mybir as mybir
    import concourse.tile as tile
    from concourse import bacc, bass_isa
    from concourse.bass import ds, ts

    f32 = mybir.dt.float32
    bf16 = mybir.dt.bfloat16
    fp16 = mybir.dt.float16
    AF = mybir.ActivationFunctionType
    ALU = mybir.AluOpType

    nc = bacc.Bacc(None, target_bir_lowering=False, num_devices=n_cores)

    # ---- kernel I/O -------------------------------------------------------
    # xpk: per-core pack; [:, 0:KO1, :] = qx^T (int-valued), [:, KO1:2*KO1, :] = x^T bf16
    xpk_t = nc.dram_tensor("xpk", [128, 2 * KO1, TC], bf16, kind="ExternalInput")
    wfc_t = nc.dram_tensor("wfc", [MO1, 128, KO1, 512], bf16, kind="ExternalInput")
    wpj_t = nc.dram_tensor("wpj", [NO2, 128, KO2, 512], bf16, kind="ExternalInput")
    # apk: [:, 0:KO1, :] = A_fc^T-tiles, [:, KO1:KO1+KO2, :] = A_proj^T-tiles
    apk_t = nc.dram_tensor("apk", [128, KO1 + KO2, R], bf16, kind="ExternalInput")
    # bpk: [:, 0:DFF] = B_fc, [:, DFF:DFF+D] = B_proj
    bpk_t = nc.dram_tensor("bpk", [R, DFF + D], bf16, kind="ExternalInput")
    # fpk: [:, 0:M64] biasfc (col mi), [:, M64:M64+D] biasproj, [:, M64+D:] scal
    # scal columns: 0: s1 = s_x*s_wfc, 1: c1 = 2/s1, 2: s_wproj, 3: unused
    fpk_t = nc.dram_tensor("fpk", [128, M64 + D + 4], f32, kind="ExternalInput")
    out_t = nc.dram_tensor("out", [MT, 128, D], f32, kind="ExternalOutput")

    SC = M64 + D  # scal column base in fpk

    with tile.TileContext(nc) as tc:
        with ExitStack() as ctx:
            consts = ctx.enter_context(tc.tile_pool(name="consts", bufs=1))
            dram = ctx.enter_context(tc.tile_pool(name="dram", bufs=1, space="DRAM"))

            # whole-kernel residents
            fpk_sb = consts.tile([128, M64 + D + 4], f32)
            apk_sb = consts.tile([128, KO1 + KO2, R], bf16)
            bpj_sb = consts.tile([R, D], bf16)
            h_sb = consts.tile([128, KO2, TC], fp16)
            xa2_sb = consts.tile([R, TC], bf16)
            maxcol = consts.tile([128, M64], f32)
            ar_in = dram.tile([128, 1], f32)
            ar_out = dram.tile(
                [128, 1], f32, addr_space="Shared" if n_cores > 4 else "Local"
            )
            nc.scalar.dma_start(fpk_sb[:], fpk_t[:])
            nc.scalar.dma_start(apk_sb[:], apk_t[:])
            nc.scalar.dma_start(bpj_sb[:], bpk_t[:, ds(DFF, D)])
            scal_sb = fpk_sb[:, ds(SC, 4)]

            # ---- phase 1: h = Gelu(s1 * (qx@qW + lora1/s1) + b_fc) -----------
            with tc.tile_pool(name="ph1c", bufs=1) as ph1c, tc.tile_pool(
                name="wfc", bufs=3
            ) as wp, tc.tile_pool(name="ps1", bufs=2, space="PSUM") as pp:
                xpk_sb = ph1c.tile([128, 2 * KO1, TC], bf16)
                bfc_sb = ph1c.tile([R, DFF], bf16)
                xa1_sb = ph1c.tile([R, TC], bf16)
                # sync queue: qx chunks interleaved with the first weight
                # halves so the first matmul isn't stuck behind the whole x
                # pack; scalar queue: lora consts + unquantized x.
                w_mo0 = wp.tile([128, KO1, 512], bf16, tag="wfc", name="w_mo")
                nc.sync.dma_start(xpk_sb[:, ds(0, 4), :], xpk_t[:, ds(0, 4), :])
                nc.sync.dma_start(w_mo0[:, ds(0, 8), :], wfc_t[0, :, ds(0, 8), :])
                nc.sync.dma_start(xpk_sb[:, ds(4, 4), :], xpk_t[:, ds(4, 4), :])
                nc.sync.dma_start(w_mo0[:, ds(8, 8), :], wfc_t[0, :, ds(8, 8), :])
                for c in range(2, 4):
                    nc.sync.dma_start(
                        xpk_sb[:, ds(4 * c, 4), :], xpk_t[:, ds(4 * c, 4), :]
                    )
                nc.scalar.dma_start(bfc_sb[:], bpk_t[:, ds(0, DFF)])
                for c in range(2):
                    nc.scalar.dma_start(
                        xpk_sb[:, ds(KO1 + 8 * c, 8), :],
                        xpk_t[:, ds(KO1 + 8 * c, 8), :],
                    )

                for mo in range(MO1):
                    if mo == 0:
                        w_mo = w_mo0
                    else:
                        w_mo = wp.tile([128, KO1, 512], bf16, tag="wfc", name="w_mo")
                        for half in range(2):
                            nc.sync.dma_start(
                                w_mo[:, ds(8 * half, 8), :],
                                wfc_t[mo, :, ds(8 * half, 8), :],
                            )
                    pss = [
                        pp.tile([128, TC], f32, tag=f"ps1_{i}", name="ps1t")
                        for i in range(4)
                    ]
                    for ko in range(KO1):
                        for sub in range(4):
                            nc.tensor.matmul(
                                pss[sub][:],
                                w_mo[:, ko, ts(sub, 128)],
                                xpk_sb[:, ko, :],
                                start=(ko == 0),
                                stop=False,
                            )
                    if mo == 0:
                        # xa1 = bf16((x @ A_fc)^T * (2/s1))
                        ps_a1 = pp.tile([128, TC], f32, tag="ps1_0", name="psa1")
                        for ko in range(KO1):
                            nc.tensor.matmul(
                                ps_a1[:R, :],
                                apk_sb[:, ko, :],
                                xpk_sb[:, KO1 + ko, :],
                                start=(ko == 0),
                                stop=(ko == KO1 - 1),
                            )
                        nc.vector.tensor_scalar_mul(
                            xa1_sb[:], ps_a1[:R, :], scal_sb[:R, 1:2]
                        )
                    for sub in range(4):
                        mi = 4 * mo + sub
                        nc.tensor.matmul(
                            pss[sub][:],
                            bfc_sb[:, ts(mi, 128)],
                            xa1_sb[:],
                            start=False,
                            stop=True,
                        )
                        nc.scalar.activation(
                            h_sb[:, mi, :],
                            pss[sub][:],
                            AF.Gelu,
                            bias=fpk_sb[:, mi : mi + 1],
                            scale=scal_sb[:, 0:1],
                        )
                        nc.vector.tensor_reduce(
                            maxcol[:, mi : mi + 1],
                            h_sb[:, mi, :],
                            axis=mybir.AxisListType.X,
                            op=ALU.max,
                            apply_absolute_value=True,
                        )

            # ---- phase 1.5: global scale via AllReduce(max) ------------------
            pmax = consts.tile([128, 1], f32)
            nc.vector.tensor_reduce(
                pmax[:], maxcol[:], axis=mybir.AxisListType.X, op=ALU.max
            )
            armax = consts.tile([128, 1], f32)
            if "no_collective" in flags:
                nc.vector.tensor_copy(armax[:], pmax[:])
            else:
                nc.gpsimd.dma_start(ar_in[:], pmax[:])
                nc.gpsimd.collective_compute(
                    "AllReduce",
                    ALU.max,
                    replica_groups=[list(range(n_cores))],
                    ins=[ar_in[:]],
                    outs=[ar_out[:]],
                )
                nc.gpsimd.dma_start(armax[:], ar_out[:])
            gmax = consts.tile([128, 1], f32)
            if "no_par_reduce" in flags:
                nc.vector.tensor_copy(gmax[:], armax[:])
            else:
                nc.gpsimd.partition_all_reduce(
                    gmax[:], armax[:], channels=128, reduce_op=bass_isa.ReduceOp.max
                )
            scaleh = consts.tile([128, 1], f32)
            invsh = consts.tile([128, 1], f32)
            s2v = consts.tile([128, 1], f32)
            c2v = consts.tile([128, 1], f32)
            nc.vector.tensor_scalar_max(gmax[:], gmax[:], 1e-8)
            # scale_h = gmax / 127  (multiply by fp32(1/127): <=1ulp from divide)
            nc.vector.tensor_scalar_mul(
                scaleh[:], gmax[:], float(np.float32(1.0) / np.float32(127.0))
            )
            nc.vector.reciprocal(invsh[:], scaleh[:])
            nc.vector.tensor_tensor(s2v[:], scaleh[:], scal_sb[:, 2:3], op=ALU.mult)
            nc.vector.reciprocal(c2v[:], s2v[:])
            nc.vector.tensor_scalar_mul(c2v[:], c2v[:], 2.0)

            # ---- phase 2: out = s2 * (qh@qW2 + lora2/s2) + b_proj ------------
            # xa2's 64 s_h-independent matmuls cross the AllReduce barrier on
            # the PE while qh production (ACT+DVE) waits for s_h.
            with tc.tile_pool(name="qh", bufs=1) as qhp, tc.tile_pool(
                name="w2", bufs=3
            ) as w2p, tc.tile_pool(name="qt", bufs=3) as qtp, tc.tile_pool(
                name="ps2", bufs=2, space="PSUM"
            ) as pp2, tc.tile_pool(name="ot", bufs=4) as otp:
                qh_sb = qhp.tile([128, KO2, TC], bf16)
                ps_a2 = pp2.tile([128, TC], f32, tag="ps2_0", name="psa2")
                w2_first = []
                for ko in range(KO2):
                    if ko % W2CH == 0:
                        w2_sb = w2p.tile([128, W2CH, 512], bf16, tag="w2", name="w2c")
                        nc.sync.dma_start(
                            w2_sb[:], wpj_t[0, :, ds(ko, W2CH), :]
                        )
                        w2_first.append(w2_sb)
                    nc.tensor.matmul(
                        ps_a2[:R, :],
                        apk_sb[:, KO1 + ko, :],
                        h_sb[:, ko, :],
                        start=(ko == 0),
                        stop=(ko == KO2 - 1),
                    )
                    qt = qtp.tile([128, TC], f32, tag="qt", name="qt")
                    nc.scalar.activation(
                        qt[:], h_sb[:, ko, :], AF.Copy, bias=0.0, scale=invsh[:, 0:1]
                    )
                    nc.vector.tensor_scalar(
                        qh_sb[:, ko, :],
                        qt[:],
                        MAGIC,
                        MAGIC,
                        op0=ALU.add,
                        op1=ALU.subtract,
                    )
                nc.vector.tensor_scalar_mul(xa2_sb[:], ps_a2[:R, :], c2v[:R, 0:1])

                for no in range(NO2):
                    ps_list = [
                        pp2.tile([128, 512], f32, tag=f"ps2_{mi}", name="ps2t")
                        for mi in range(MT)
                    ]
                    for ko in range(KO2):
                        ch = ko // W2CH
                        if no == 0:
                            w2_sb = w2_first[ch]
                        elif ko % W2CH == 0:
                            w2_sb = w2p.tile(
                                [128, W2CH, 512], bf16, tag="w2", name="w2c"
                            )
                            nc.sync.dma_start(
                                w2_sb[:], wpj_t[no, :, ds(ko, W2CH), :]
                            )
                        for mi in range(MT):
                            nc.tensor.matmul(
                                ps_list[mi][:],
                                qh_sb[:, ko, ts(mi, 128)],
                                w2_sb[:, ko % W2CH, :],
                                start=(ko == 0),
                                stop=False,
                            )
                    for mi in range(MT):
                        nc.tensor.matmul(
                            ps_list[mi][:],
                            xa2_sb[:, ts(mi, 128)],
                            bpj_sb[:, ds(no * 512, 512)],
                            start=False,
                            stop=True,
                        )
                        ot = otp.tile([128, 512], f32, tag="ot", name="ot")
                        # scale on ACT, bias-add on DVE (halves eviction latency
                        # at psum-bank reuse boundaries)
                        nc.scalar.activation(
                            ot[:], ps_list[mi][:], AF.Copy, bias=0.0, scale=s2v[:, 0:1]
                        )
                        nc.vector.tensor_add(
                            ot[:], ot[:], fpk_sb[:, ds(M64 + no * 512, 512)]
                        )
                        nc.scalar.dma_start(out_t[mi, :, ds(no * 512, 512)], ot[:])

    nc.compile()
    return nc


def _scale_of(a):
    m = np.max(np.abs(a)).astype(np.float32)
    m = np.maximum(m, np.float32(1e-8))
    return (m / QMAX).astype(np.float32)


def _quant(a, s):
    return np.clip(np.round(a / s), -QMAX, QMAX)


def _prep_weights(W_fc, b_fc, A_fc, B_fc, W_proj, b_proj, A_proj, B_proj):
    """Shared (replicated) input tensors from the weight arrays."""
    bf16 = ml_dtypes.bfloat16
    W_fc = np.asarray(W_fc, np.float32)
    W_proj = np.asarray(W_proj, np.float32)

    s_wfc = _scale_of(W_fc)
    s_wp = _scale_of(W_proj)
    qwfc = _quant(W_fc, s_wfc)
    qwp = _quant(W_proj, s_wp)

    # [k, ff] -> [mo, p(k%128), ko, c(ff%512)]
    wfc_dev = np.ascontiguousarray(
        qwfc.reshape(KO1, 128, MO1, 512).transpose(2, 1, 0, 3).astype(bf16)
    )
    # [k, d] -> [no, p(k%128), ko, c(d%512)]
    wpj_dev = np.ascontiguousarray(
        qwp.reshape(KO2, 128, NO2, 512).transpose(2, 1, 0, 3).astype(bf16)
    )
    afc = np.asarray(A_fc, np.float32).reshape(KO1, 128, R).transpose(1, 0, 2)
    apj = np.asarray(A_proj, np.float32).reshape(KO2, 128, R).transpose(1, 0, 2)
    apk_dev = np.ascontiguousarray(
        np.concatenate([afc, apj], axis=1).astype(bf16)
    )
    bpk_dev = np.ascontiguousarray(
        np.concatenate(
            [np.asarray(B_fc, np.float32), np.asarray(B_proj, np.float32)], axis=1
        ).astype(bf16)
    )
    fpk_dev = np.empty((128, M64 + D + 4), np.float32)
    fpk_dev[:, 0:M64] = np.asarray(b_fc, np.float32).reshape(M64, 128).T
    fpk_dev[:, M64 : M64 + D] = np.asarray(b_proj, np.float32)[None, :]
    return {
        "wfc": wfc_dev,
        "wpj": wpj_dev,
        "apk": apk_dev,
        "bpk": bpk_dev,
        "fpk": fpk_dev,
        "s_wfc": s_wfc,
        "s_wp": s_wp,
    }


def _prep_inputs(hidden_states, W_fc, b_fc, A_fc, B_fc, W_proj, b_proj, A_proj, B_proj):
    bf16 = ml_dtypes.bfloat16
    w = _prep_weights(W_fc, b_fc, A_fc, B_fc, W_proj, b_proj, A_proj, B_proj)
    x = np.ascontiguousarray(np.asarray(hidden_states, np.float32).reshape(T, D))
    s_x = _scale_of(x)
    qx = _quant(x, s_x)
    s1 = np.float32(s_x * w["s_wfc"])
    c1 = np.float32(np.float32(2.0) / s1)
    fpk = w["fpk"].copy()
    fpk[:, M64 + D :] = np.array([s1, c1, w["s_wp"], 0.0], np.float32)[None, :]

    shared = {"wfc": w["wfc"], "wpj": w["wpj"], "apk": w["apk"], "bpk": w["bpk"], "fpk": fpk}
    in_maps = []
    for c in range(NCORES):
        xc = x[c * TC : (c + 1) * TC]  # [TC, D]
        qxc = qx[c * TC : (c + 1) * TC]
        qxT = qxc.T.reshape(KO1, 128, TC).transpose(1, 0, 2)
        xT = xc.T.reshape(KO1, 128, TC).transpose(1, 0, 2)
        xpk = np.ascontiguousarray(
            np.concatenate([qxT, xT], axis=1).astype(bf16)
        )
        in_maps.append({**shared, "xpk": xpk})
    return in_maps


def _get_runner(**build_kwargs):
    """Build the Bass program once and wrap it in a cached jitted shard_map
    executable (adapted from concourse.bass2jax.run_bass_via_pjrt)."""
    key = ("runner", tuple(sorted(build_kwargs.items())))
    if key in _CACHE:
        return _CACHE[key]
    nc = _build_nc(**build_kwargs)
    n_cores_ = build_kwargs.get("n_cores", NCORES)
    runner = _runner_from_nc(nc, n_cores_)
    _CACHE[key] = runner
    return runner


def _runner_from_nc(nc, n_cores_):
    import jax
    import jax.numpy as jnp  # noqa: F401
    from jax.experimental.shard_map import shard_map
    from jax.sharding import Mesh, PartitionSpec

    from concourse import bass2jax, mybir

    bass2jax.install_neuronx_cc_hook()
    assert nc.dbg_addr is None
    partition_name = nc.partition_id_tensor.name if nc.partition_id_tensor else None

    in_names = []
    out_names = []
    out_avals = []
    for alloc in nc.m.functions[0].allocations:
        if not isinstance(alloc, mybir.MemoryLocationSet):
            continue
        name = alloc.memorylocations[0].name
        if alloc.kind == "ExternalInput":
            if name != partition_name:
                in_names.append(name)
        elif alloc.kind == "ExternalOutput":
            out_names.append(name)
            out_avals.append(
                jax.core.ShapedArray(tuple(alloc.tensor_shape), mybir.dt.np(alloc.dtype))
            )
    all_in_names = tuple(in_names) + tuple(out_names)
    if partition_name is not None:
        all_in_names = all_in_names + (partition_name,)
    n_params = len(in_names)
    n_outs = len(out_names)

    def _body(*args):
        operands = list(args)
        if partition_name is not None:
            operands.append(bass2jax.partition_id_tensor())
        outs = bass2jax._bass_exec_p.bind(
            *operands,
            out_avals=tuple(out_avals),
            in_names=all_in_names,
            out_names=tuple(out_names),
            lowering_input_output_aliases=(),
            sim_require_finite=True,
            sim_require_nnan=True,
            nc=nc,
        )
        return tuple(outs)

    devices = jax.devices()[:n_cores_]
    assert len(devices) == n_cores_, f"need {n_cores_} devices, have {len(jax.devices())}"
    mesh = Mesh(np.asarray(devices), ("core",))
    in_specs = (PartitionSpec("core"),) * (n_params + n_outs)
    out_specs = (PartitionSpec("core"),) * n_outs
    donate = tuple(range(n_params, n_params + n_outs))
    fn = jax.jit(
        shard_map(
            _body, mesh=mesh, in_specs=in_specs, out_specs=out_specs, check_rep=False
        ),
        donate_argnums=donate,
        keep_unused=True,
    )
    runner = {
        "fn": fn,
        "in_names": in_names,
        "out_names": out_names,
        "out_avals": out_avals,
        "mesh": mesh,
    }
    runner["n_cores"] = n_cores_
    return runner


def _zero_outs(runner):
    n = runner["n_cores"]
    return [
        np.zeros((n * a.shape[0], *a.shape[1:]), a.dtype) for a in runner["out_avals"]
    ]


def _concat_inputs(in_maps, in_names):
    return [
        np.concatenate([m[name] for m in in_maps], axis=0) for name in in_names
    ]


def kernel(hidden_states, W_fc, b_fc, A_fc, B_fc, W_proj, b_proj, A_proj, B_proj):
    global LAST_RESULT
    runner = _get_runner()
    in_maps = _prep_inputs(
        hidden_states, W_fc, b_fc, A_fc, B_fc, W_proj, b_proj, A_proj, B_proj
    )
    concat_in = _concat_inputs(in_maps, runner["in_names"])
    out_arrs = runner["fn"](*concat_in, *_zero_outs(runner))
    (out_global,) = [np.asarray(a) for a in out_arrs]
    # out_global: [NCORES*MT, 128, D] -> per-core [MT,128,D] -> tokens x D
    out = out_global.reshape(T, D).astype(np.float32)
    return out.reshape(B_, S, D)


def bench(n_iters=20, in_maps=None):
    """Steady-state per-iteration wall time of the compiled executable with
    device-resident inputs (upper bound on HW exec time; includes dispatch)."""
    import time

    import jax

    runner = _get_runner()
    if in_maps is None:
        rng = np.random.default_rng(0)
        dummy = {
            "hidden_states": rng.standard_normal((B_, S, D), dtype=np.float32),
            "W_fc": rng.standard_normal((D, DFF), dtype=np.float32) / 45.0,
            "b_fc": np.zeros(DFF, np.float32),
            "A_fc": rng.standard_normal((D, R), dtype=np.float32) / 45.0,
            "B_fc": rng.standard_normal((R, DFF), dtype=np.float32) * 0.01,
            "W_proj": rng.standard_normal((DFF, D), dtype=np.float32) / 90.0,
            "b_proj": np.zeros(D, np.float32),
            "A_proj": rng.standard_normal((DFF, R), dtype=np.float32) / 90.0,
            "B_proj": rng.standard_normal((R, D), dtype=np.float32) * 0.01,
        }
        in_maps = _prep_inputs(**dummy)
    concat_in = _concat_inputs(in_maps, runner["in_names"])
    from jax.sharding import NamedSharding, PartitionSpec

    sharding = NamedSharding(runner["mesh"], PartitionSpec("core"))
    dev_in = [jax.device_put(a, sharding) for a in concat_in]
    # donated output buffers are consumed per call: pre-stage one set per iter
    zero_sets = [
        [jax.device_put(z, sharding) for z in _zero_outs(runner)]
        for _ in range(n_iters + 1)
    ]
    out = runner["fn"](*dev_in, *zero_sets[-1])
    jax.block_until_ready(out)
    t0 = time.time()
    for i in range(n_iters):
        out = runner["fn"](*dev_in, *zero_sets[i])
    jax.block_until_ready(out)
    dt = (time.time() - t0) / n_iters
    return dt


# revision 23
# speedup vs baseline: 2.1721x; 1.5586x over previous
"""QLoRA-style MLP (fake-quant base + LoRA + exact GeLU) on 8 TRN2 cores.

Sharding: token data-parallel (4096 tokens / 8 cores = 512 tokens per core),
weights replicated.  The only cross-core communication is a tiny AllReduce(max)
for the global fake-quant scale of the hidden activation.

Math per layer (matching the jax reference):
    base = fq(x) @ fq(W) + b          fq(t) = clip(round(t/s), -127, 127) * s,
                                      s = max(max|t|, 1e-8) / 127  (global max)
    lora = 2.0 * (x @ A) @ B          (bf16 operands on device)
    out  = base + lora                (layer 1 additionally GeLU'd, exact erf)

v3 design (vs v1): h kept resident in SBUF as fp16 (no HBM spill/reload),
all LoRA matmuls stream bf16 (1 cycle/row on the PE instead of 4 for f32),
inputs packed into 6 DRAM tensors, weights repacked host-side so streaming
DMAs are few and large (16 x 2MB for W_fc, 32 x 1MB for W_proj).

Device mapping (per core, T=512 tokens):
  L1: psum[ff128, T] = sum_ko wfc[mo][k,ff]^T @ qx[k,T]      (bf16 int matmul)
                       + B_fc[16,ff]^T-slice @ xa1[16,T]     (bf16, K=16)
      h[ff,T](fp16) = Gelu(psum * s1 + b_fc); track per-column absmax
  AllReduce(max) -> s_h
  L2: xa2[16,T] = A_proj^T @ h  (64 bf16 matmuls, overlaps the AllReduce)
      qh[ff,T](bf16) = round(h / s_h)   (ACT scale + DVE magic-round)
      psum[tok128, 512] = sum_ko qh[k,tok]^T-tiles @ wproj[no][k,512]
                          + xa2[16,tok]^T-slice @ B_proj[16,512]
      out = psum * s2 + b_proj
"""

import os
import sys

import numpy as np

if "/opt/trn_rl_repo" not in sys.path:
    sys.path.insert(0, "/opt/trn_rl_repo")

import ml_dtypes

# Problem shapes (hardcoded per contract).
B_, S, D, DFF, R = 2, 2048, 2048, 8192, 16
T = B_ * S  # 4096 tokens
NCORES = 8
TC = T // NCORES  # 512 tokens per core
QMAX = np.float32(127.0)
MAGIC = float(np.float32(12582912.0))  # 1.5 * 2**23: fp32 round-half-even trick

KO1 = D // 128  # 16  k-tiles for layer 1
MO1 = DFF // 512  # 16  512-wide ff blocks
M64 = DFF // 128  # 64  128-wide ff blocks
KO2 = DFF // 128  # 64  k-tiles for layer 2
NO2 = D // 512  # 4   512-wide output-col blocks
MT = TC // 128  # 4   token tiles per core
W2CH = 8  # W_proj ko-tiles per streamed chunk
NCH2 = KO2 // W2CH  # 8 chunks per no

_CACHE = {}
LAST_RESULT = None


def _build_nc(n_cores=NCORES, flags=(), loop_k=None):
    """Build + compile the Bass program.

    loop_k: when set, wrap the whole forward pass in a hardware For loop that
    executes it ``loop_k`` times per NEFF launch (used by ``bench`` to measure
    steady-state device throughput without per-dispatch tunnel overhead)."""
    from contextlib import ExitStack

    import concourse.bass as bass  # noqa: F401
    import concourse.mybir as mybir
    import concourse.tile as tile
    from concourse import bacc, bass_isa
    from concourse.bass import ds, ts

    f32 = mybir.dt.float32
    bf16 = mybir.dt.bfloat16
    fp16 = mybir.dt.float16
    AF = mybir.ActivationFunctionType
    ALU = mybir.AluOpType

    nc = bacc.Bacc(None, target_bir_lowering=False, num_devices=n_cores)

    # ---- kernel I/O -------------------------------------------------------
    # xpk: per-core pack; [:, 0:KO1, :] = qx^T (int-valued), [:, KO1:2*KO1, :] = x^T bf16
    xpk_t = nc.dram_tensor("xpk", [128, 2 * KO1, TC], bf16, kind="ExternalInput")
    wfc_t = nc.dram_tensor("wfc", [MO1, 128, KO1, 512], bf16, kind="ExternalInput")
    wpj_t = nc.dram_tensor("wpj", [NO2, 128, KO2, 512], bf16, kind="ExternalInput")
    # apk: [:, 0:KO1, :] = A_fc^T-tiles, [:, KO1:KO1+KO2, :] = A_proj^T-tiles
    apk_t = nc.dram_tensor("apk", [128, KO1 + KO2, R], bf16, kind="ExternalInput")
    # bpk: [:, 0:DFF] = B_fc, [:, DFF:DFF+D] = B_proj
    bpk_t = nc.dram_tensor("bpk", [R, DFF + D], bf16, kind="ExternalInput")
    # fpk: [:, 0:M64] biasfc (col mi), [:, M64:M64+D] biasproj, [:, M64+D:] scal
    # scal columns: 0: s1 = s_x*s_wfc, 1: c1 = 2/s1, 2: s_wproj, 3: unused
    fpk_t = nc.dram_tensor("fpk", [128, M64 + D + 4], f32, kind="ExternalInput")
    out_t = nc.dram_tensor("out", [MT, 128, D], f32, kind="ExternalOutput")

    SC = M64 + D  # scal column base in fpk
    p1 = "phase2_only" not in flags
    p2 = "phase1_only" not in flags

    with tile.TileContext(nc) as tc:
        with ExitStack() as ctx:
            consts = ctx.enter_context(tc.tile_pool(name="consts", bufs=1))
            dram = ctx.enter_context(tc.tile_pool(name="dram", bufs=1, space="DRAM"))

            # whole-kernel residents
            fpk_sb = consts.tile([128, M64 + D + 4], f32)
            apk_sb = consts.tile([128, KO1 + KO2, R], bf16)
            bpj_sb = consts.tile([R, D], bf16)
            h_sb = consts.tile([128, KO2, TC], fp16)
            xa2_sb = consts.tile([R, TC], bf16)
            maxcol = consts.tile([128, M64], f32)
            ar_in = dram.tile([128, 1], f32)
            ar_out = dram.tile(
                [128, 1], f32, addr_space="Shared" if n_cores > 4 else "Local"
            )
            nc.scalar.dma_start(fpk_sb[:], fpk_t[:])
            nc.scalar.dma_start(apk_sb[:], apk_t[:])
            nc.scalar.dma_start(bpj_sb[:], bpk_t[:, ds(DFF, D)])
            scal_sb = fpk_sb[:, ds(SC, 4)]

            if loop_k:
                ctx.enter_context(tc.For_i(0, loop_k, 1))

            # ---- phase 1: h = Gelu(s1 * (qx@qW + lora1/s1) + b_fc) -----------
            with tc.tile_pool(name="ph1c", bufs=1) as ph1c, tc.tile_pool(
                name="wfc", bufs=3
            ) as wp, tc.tile_pool(name="ps1", bufs=2, space="PSUM") as pp:
                xpk_sb = ph1c.tile([128, 2 * KO1, TC], bf16)
                bfc_sb = ph1c.tile([R, DFF], bf16)
                xa1_sb = ph1c.tile([R, TC], bf16)
                if p1:
                    # sync queue: qx chunks interleaved with the first weight
                    # halves so the first matmul isn't stuck behind the whole x
                    # pack; scalar queue: lora consts + unquantized x.
                    w_mo0 = wp.tile([128, KO1, 512], bf16, tag="wfc", name="w_mo")
                    nc.sync.dma_start(xpk_sb[:, ds(0, 4), :], xpk_t[:, ds(0, 4), :])
                    nc.sync.dma_start(w_mo0[:, ds(0, 8), :], wfc_t[0, :, ds(0, 8), :])
                    nc.sync.dma_start(xpk_sb[:, ds(4, 4), :], xpk_t[:, ds(4, 4), :])
                    nc.sync.dma_start(w_mo0[:, ds(8, 8), :], wfc_t[0, :, ds(8, 8), :])
                    for c in range(2, 4):
                        nc.sync.dma_start(
                            xpk_sb[:, ds(4 * c, 4), :], xpk_t[:, ds(4 * c, 4), :]
                        )
                    nc.scalar.dma_start(bfc_sb[:], bpk_t[:, ds(0, DFF)])
                    for c in range(2):
                        nc.scalar.dma_start(
                            xpk_sb[:, ds(KO1 + 8 * c, 8), :],
                            xpk_t[:, ds(KO1 + 8 * c, 8), :],
                        )

                for mo in range(MO1 if p1 else 0):
                    if mo == 0:
                        w_mo = w_mo0
                    else:
                        w_mo = wp.tile([128, KO1, 512], bf16, tag="wfc", name="w_mo")
                        for half in range(2):
                            nc.sync.dma_start(
                                w_mo[:, ds(8 * half, 8), :],
                                wfc_t[mo, :, ds(8 * half, 8), :],
                            )
                    pss = [
                        pp.tile([128, TC], f32, tag=f"ps1_{i}", name="ps1t")
                        for i in range(4)
                    ]
                    for ko in range(KO1):
                        for sub in range(4):
                            nc.tensor.matmul(
                                pss[sub][:],
                                w_mo[:, ko, ts(sub, 128)],
                                xpk_sb[:, ko, :],
                                start=(ko == 0),
                                stop=False,
                            )
                    if mo == 0:
                        # xa1 = bf16((x @ A_fc)^T * (2/s1))
                        ps_a1 = pp.tile([128, TC], f32, tag="ps1_0", name="psa1")
                        for ko in range(KO1):
                            nc.tensor.matmul(
                                ps_a1[:R, :],
                                apk_sb[:, ko, :],
                                xpk_sb[:, KO1 + ko, :],
                                start=(ko == 0),
                                stop=(ko == KO1 - 1),
                            )
                        nc.vector.tensor_scalar_mul(
                            xa1_sb[:], ps_a1[:R, :], scal_sb[:R, 1:2]
                        )
                    for sub in range(4):
                        mi = 4 * mo + sub
                        nc.tensor.matmul(
                            pss[sub][:],
                            bfc_sb[:, ts(mi, 128)],
                            xa1_sb[:],
                            start=False,
                            stop=True,
                        )
                        nc.scalar.activation(
                            h_sb[:, mi, :],
                            pss[sub][:],
                            AF.Gelu,
                            bias=fpk_sb[:, mi : mi + 1],
                            scale=scal_sb[:, 0:1],
                        )
                        nc.vector.tensor_reduce(
                            maxcol[:, mi : mi + 1],
                            h_sb[:, mi, :],
                            axis=mybir.AxisListType.X,
                            op=ALU.max,
                            apply_absolute_value=True,
                        )

            # ---- phase 1.5: global scale via AllReduce(max) ------------------
            if p2:
                if not p1:
                    nc.vector.memset(maxcol[:], 1.0)
                    nc.vector.memset(h_sb[:], 0)
                pmax = consts.tile([128, 1], f32)
                nc.vector.tensor_reduce(
                    pmax[:], maxcol[:], axis=mybir.AxisListType.X, op=ALU.max
                )
                armax = consts.tile([128, 1], f32)
                if "no_collective" in flags:
                    nc.vector.tensor_copy(armax[:], pmax[:])
                else:
                    nc.gpsimd.dma_start(ar_in[:], pmax[:])
                    nc.gpsimd.collective_compute(
                        "AllReduce",
                        ALU.max,
                        replica_groups=[list(range(n_cores))],
                        ins=[ar_in[:]],
                        outs=[ar_out[:]],
                    )
                    nc.gpsimd.dma_start(armax[:], ar_out[:])
                gmax = consts.tile([128, 1], f32)
                if "no_par_reduce" in flags:
                    nc.vector.tensor_copy(gmax[:], armax[:])
                else:
                    nc.gpsimd.partition_all_reduce(
                        gmax[:], armax[:], channels=128, reduce_op=bass_isa.ReduceOp.max
                    )
                scaleh = consts.tile([128, 1], f32)
                invsh = consts.tile([128, 1], f32)
                s2v = consts.tile([128, 1], f32)
                c2v = consts.tile([128, 1], f32)
                nc.vector.tensor_scalar_max(gmax[:], gmax[:], 1e-8)
                # scale_h = gmax / 127  (mult by fp32(1/127): <=1ulp from divide)
                nc.vector.tensor_scalar_mul(
                    scaleh[:], gmax[:], float(np.float32(1.0) / np.float32(127.0))
                )
                nc.vector.reciprocal(invsh[:], scaleh[:])
                nc.vector.tensor_tensor(s2v[:], scaleh[:], scal_sb[:, 2:3], op=ALU.mult)
                nc.vector.reciprocal(c2v[:], s2v[:])
                nc.vector.tensor_scalar_mul(c2v[:], c2v[:], 2.0)

            # ---- phase 2: out = s2 * (qh@qW2 + lora2/s2) + b_proj ------------
            # xa2's 64 s_h-independent matmuls cross the AllReduce barrier on
            # the PE while qh production (ACT+DVE) waits for s_h.
            if not p2:
                pass
            else:
             with tc.tile_pool(name="qh", bufs=1) as qhp, tc.tile_pool(
                name="w2", bufs=3
            ) as w2p, tc.tile_pool(name="qt", bufs=3) as qtp, tc.tile_pool(
                name="ps2", bufs=2, space="PSUM"
            ) as pp2, tc.tile_pool(name="ot", bufs=4) as otp:
                qh_sb = qhp.tile([128, KO2, TC], bf16)
                ps_a2 = pp2.tile([128, TC], f32, tag="ps2_0", name="psa2")
                w2_first = []
                for ko in range(KO2):
                    if ko % W2CH == 0:
                        w2_sb = w2p.tile([128, W2CH, 512], bf16, tag="w2", name="w2c")
                        nc.sync.dma_start(
                            w2_sb[:], wpj_t[0, :, ds(ko, W2CH), :]
                        )
                        w2_first.append(w2_sb)
                    nc.tensor.matmul(
                        ps_a2[:R, :],
                        apk_sb[:, KO1 + ko, :],
                        h_sb[:, ko, :],
                        start=(ko == 0),
                        stop=(ko == KO2 - 1),
                    )
                    qt = qtp.tile([128, TC], f32, tag="qt", name="qt")
                    nc.scalar.activation(
                        qt[:], h_sb[:, ko, :], AF.Copy, bias=0.0, scale=invsh[:, 0:1]
                    )
                    nc.vector.tensor_scalar(
                        qh_sb[:, ko, :],
                        qt[:],
                        MAGIC,
                        MAGIC,
                        op0=ALU.add,
                        op1=ALU.subtract,
                    )
                nc.vector.tensor_scalar_mul(xa2_sb[:], ps_a2[:R, :], c2v[:R, 0:1])

                for no in range(NO2):
                    ps_list = [
                        pp2.tile([128, 512], f32, tag=f"ps2_{mi}", name="ps2t")
                        for mi in range(MT)
                    ]
                    for ko in range(KO2):
                        ch = ko // W2CH
                        if no == 0:
                            w2_sb = w2_first[ch]
                        elif ko % W2CH == 0:
                            w2_sb = w2p.tile(
                                [128, W2CH, 512], bf16, tag="w2", name="w2c"
                            )
                            nc.sync.dma_start(
                                w2_sb[:], wpj_t[no, :, ds(ko, W2CH), :]
                            )
                        for mi in range(MT):
                            nc.tensor.matmul(
                                ps_list[mi][:],
                                qh_sb[:, ko, ts(mi, 128)],
                                w2_sb[:, ko % W2CH, :],
                                start=(ko == 0),
                                stop=False,
                            )
                    for mi in range(MT):
                        nc.tensor.matmul(
                            ps_list[mi][:],
                            xa2_sb[:, ts(mi, 128)],
                            bpj_sb[:, ds(no * 512, 512)],
                            start=False,
                            stop=True,
                        )
                        ot = otp.tile([128, 512], f32, tag="ot", name="ot")
                        # scale on ACT, bias-add on DVE (halves eviction latency
                        # at psum-bank reuse boundaries)
                        nc.scalar.activation(
                            ot[:], ps_list[mi][:], AF.Copy, bias=0.0, scale=s2v[:, 0:1]
                        )
                        nc.vector.tensor_add(
                            ot[:], ot[:], fpk_sb[:, ds(M64 + no * 512, 512)]
                        )
                        nc.scalar.dma_start(out_t[mi, :, ds(no * 512, 512)], ot[:])

    nc.compile()
    return nc


def _scale_of(a):
    m = np.max(np.abs(a)).astype(np.float32)
    m = np.maximum(m, np.float32(1e-8))
    return (m / QMAX).astype(np.float32)


def _quant(a, s):
    return np.clip(np.round(a / s), -QMAX, QMAX)


def _prep_weights(W_fc, b_fc, A_fc, B_fc, W_proj, b_proj, A_proj, B_proj):
    """Shared (replicated) input tensors from the weight arrays."""
    bf16 = ml_dtypes.bfloat16
    W_fc = np.asarray(W_fc, np.float32)
    W_proj = np.asarray(W_proj, np.float32)

    s_wfc = _scale_of(W_fc)
    s_wp = _scale_of(W_proj)
    qwfc = _quant(W_fc, s_wfc)
    qwp = _quant(W_proj, s_wp)

    # [k, ff] -> [mo, p(k%128), ko, c(ff%512)]
    wfc_dev = np.ascontiguousarray(
        qwfc.reshape(KO1, 128, MO1, 512).transpose(2, 1, 0, 3).astype(bf16)
    )
    # [k, d] -> [no, p(k%128), ko, c(d%512)]
    wpj_dev = np.ascontiguousarray(
        qwp.reshape(KO2, 128, NO2, 512).transpose(2, 1, 0, 3).astype(bf16)
    )
    afc = np.asarray(A_fc, np.float32).reshape(KO1, 128, R).transpose(1, 0, 2)
    apj = np.asarray(A_proj, np.float32).reshape(KO2, 128, R).transpose(1, 0, 2)
    apk_dev = np.ascontiguousarray(
        np.concatenate([afc, apj], axis=1).astype(bf16)
    )
    bpk_dev = np.ascontiguousarray(
        np.concatenate(
            [np.asarray(B_fc, np.float32), np.asarray(B_proj, np.float32)], axis=1
        ).astype(bf16)
    )
    fpk_dev = np.empty((128, M64 + D + 4), np.float32)
    fpk_dev[:, 0:M64] = np.asarray(b_fc, np.float32).reshape(M64, 128).T
    fpk_dev[:, M64 : M64 + D] = np.asarray(b_proj, np.float32)[None, :]
    return {
        "wfc": wfc_dev,
        "wpj": wpj_dev,
        "apk": apk_dev,
        "bpk": bpk_dev,
        "fpk": fpk_dev,
        "s_wfc": s_wfc,
        "s_wp": s_wp,
    }


def _prep_inputs(hidden_states, W_fc, b_fc, A_fc, B_fc, W_proj, b_proj, A_proj, B_proj):
    bf16 = ml_dtypes.bfloat16
    w = _prep_weights(W_fc, b_fc, A_fc, B_fc, W_proj, b_proj, A_proj, B_proj)
    x = np.ascontiguousarray(np.asarray(hidden_states, np.float32).reshape(T, D))
    s_x = _scale_of(x)
    qx = _quant(x, s_x)
    s1 = np.float32(s_x * w["s_wfc"])
    c1 = np.float32(np.float32(2.0) / s1)
    fpk = w["fpk"].copy()
    fpk[:, M64 + D :] = np.array([s1, c1, w["s_wp"], 0.0], np.float32)[None, :]

    shared = {"wfc": w["wfc"], "wpj": w["wpj"], "apk": w["apk"], "bpk": w["bpk"], "fpk": fpk}
    in_maps = []
    for c in range(NCORES):
        xc = x[c * TC : (c + 1) * TC]  # [TC, D]
        qxc = qx[c * TC : (c + 1) * TC]
        qxT = qxc.T.reshape(KO1, 128, TC).transpose(1, 0, 2)
        xT = xc.T.reshape(KO1, 128, TC).transpose(1, 0, 2)
        xpk = np.ascontiguousarray(
            np.concatenate([qxT, xT], axis=1).astype(bf16)
        )
        in_maps.append({**shared, "xpk": xpk})
    return in_maps


def _get_runner(**build_kwargs):
    """Build the Bass program once and wrap it in a cached jitted shard_map
    executable (adapted from concourse.bass2jax.run_bass_via_pjrt)."""
    key = ("runner", tuple(sorted(build_kwargs.items())))
    if key in _CACHE:
        return _CACHE[key]
    nc = _build_nc(**build_kwargs)
    n_cores_ = build_kwargs.get("n_cores", NCORES)
    runner = _runner_from_nc(nc, n_cores_)
    _CACHE[key] = runner
    return runner


def _runner_from_nc(nc, n_cores_):
    import jax
    import jax.numpy as jnp  # noqa: F401
    from jax.experimental.shard_map import shard_map
    from jax.sharding import Mesh, PartitionSpec

    from concourse import bass2jax, mybir

    bass2jax.install_neuronx_cc_hook()
    assert nc.dbg_addr is None
    partition_name = nc.partition_id_tensor.name if nc.partition_id_tensor else None

    in_names = []
    out_names = []
    out_avals = []
    for alloc in nc.m.functions[0].allocations:
        if not isinstance(alloc, mybir.MemoryLocationSet):
            continue
        name = alloc.memorylocations[0].name
        if alloc.kind == "ExternalInput":
            if name != partition_name:
                in_names.append(name)
        elif alloc.kind == "ExternalOutput":
            out_names.append(name)
            out_avals.append(
                jax.core.ShapedArray(tuple(alloc.tensor_shape), mybir.dt.np(alloc.dtype))
            )
    all_in_names = tuple(in_names) + tuple(out_names)
    if partition_name is not None:
        all_in_names = all_in_names + (partition_name,)
    n_params = len(in_names)
    n_outs = len(out_names)

    def _body(*args):
        operands = list(args)
        if partition_name is not None:
            operands.append(bass2jax.partition_id_tensor())
        outs = bass2jax._bass_exec_p.bind(
            *operands,
            out_avals=tuple(out_avals),
            in_names=all_in_names,
            out_names=tuple(out_names),
            lowering_input_output_aliases=(),
            sim_require_finite=True,
            sim_require_nnan=True,
            nc=nc,
        )
        return tuple(outs)

    devices = jax.devices()[:n_cores_]
    assert len(devices) == n_cores_, f"need {n_cores_} devices, have {len(jax.devices())}"
    mesh = Mesh(np.asarray(devices), ("core",))
    in_specs = (PartitionSpec("core"),) * (n_params + n_outs)
    out_specs = (PartitionSpec("core"),) * n_outs
    donate = tuple(range(n_params, n_params + n_outs))
    fn = jax.jit(
        shard_map(
            _body, mesh=mesh, in_specs=in_specs, out_specs=out_specs, check_rep=False
        ),
        donate_argnums=donate,
        keep_unused=True,
    )
    runner = {
        "fn": fn,
        "in_names": in_names,
        "out_names": out_names,
        "out_avals": out_avals,
        "mesh": mesh,
        "nc": nc,
    }
    runner["n_cores"] = n_cores_
    return runner


def _zero_outs(runner):
    n = runner["n_cores"]
    return [
        np.zeros((n * a.shape[0], *a.shape[1:]), a.dtype) for a in runner["out_avals"]
    ]


def _concat_inputs(in_maps, in_names):
    return [
        np.concatenate([m[name] for m in in_maps], axis=0) for name in in_names
    ]


def kernel(hidden_states, W_fc, b_fc, A_fc, B_fc, W_proj, b_proj, A_proj, B_proj):
    global LAST_RESULT
    runner = _get_runner()
    in_maps = _prep_inputs(
        hidden_states, W_fc, b_fc, A_fc, B_fc, W_proj, b_proj, A_proj, B_proj
    )
    concat_in = _concat_inputs(in_maps, runner["in_names"])
    out_arrs = runner["fn"](*concat_in, *_zero_outs(runner))
    (out_global,) = [np.asarray(a) for a in out_arrs]
    # out_global: [NCORES*MT, 128, D] -> per-core [MT,128,D] -> tokens x D
    out = out_global.reshape(T, D).astype(np.float32)
    return out.reshape(B_, S, D)


def _dummy_in_maps():
    rng = np.random.default_rng(0)
    dummy = {
        "hidden_states": rng.standard_normal((B_, S, D), dtype=np.float32),
        "W_fc": rng.standard_normal((D, DFF), dtype=np.float32) / 45.0,
        "b_fc": np.zeros(DFF, np.float32),
        "A_fc": rng.standard_normal((D, R), dtype=np.float32) / 45.0,
        "B_fc": rng.standard_normal((R, DFF), dtype=np.float32) * 0.01,
        "W_proj": rng.standard_normal((DFF, D), dtype=np.float32) / 90.0,
        "b_proj": np.zeros(D, np.float32),
        "A_proj": rng.standard_normal((DFF, R), dtype=np.float32) / 90.0,
        "B_proj": rng.standard_normal((R, D), dtype=np.float32) * 0.01,
    }
    return _prep_inputs(**dummy)


def bench(n_iters=20, in_maps=None, rounds=3, **build_kwargs):
    """Per-iteration wall time of the full kernel (one dispatch per forward
    pass, device-resident inputs), best of ``rounds`` runs of ``n_iters``
    iterations each.  The axon tunnel's dispatch cost degrades one-sidedly
    over a session, so min-of-rounds is the robust steady-state estimate."""
    import time

    import jax
    from jax.sharding import NamedSharding, PartitionSpec

    runner = _get_runner(**build_kwargs)
    if in_maps is None:
        in_maps = _dummy_in_maps()
    concat_in = _concat_inputs(in_maps, runner["in_names"])
    sharding = NamedSharding(runner["mesh"], PartitionSpec("core"))
    dev_in = [jax.device_put(a, sharding) for a in concat_in]
    for a in dev_in:
        jax.block_until_ready(a)

    best = None
    for r in range(rounds):
        zero_sets = [
            [jax.device_put(z, sharding) for z in _zero_outs(runner)]
            for _ in range(n_iters + 1)
        ]
        for zs in zero_sets:
            for z in zs:
                jax.block_until_ready(z)
        out = runner["fn"](*dev_in, *zero_sets[-1])  # warmup (compile on r=0)
        jax.block_until_ready(out)
        t0 = time.time()
        for i in range(n_iters):
            out = runner["fn"](*dev_in, *zero_sets[i])
        jax.block_until_ready(out)
        dt = (time.time() - t0) / n_iters
        if best is None or dt < best:
            best = dt
    return best


def bench_device_loop(n_iters=20, in_maps=None, rounds=2, flags=()):
    """Device-side steady-state throughput: one dispatch executes a NEFF whose
    body is a hardware For loop running the forward pass ``n_iters`` times
    back-to-back, amortizing the per-call tunnel dispatch (~1.7ms on this
    setup).  The cross-core AllReduce desyncs the collective fabric when
    executed inside a hardware loop here, so this diagnostic runs the
    no_collective build (identical kernel minus the inter-core max exchange)."""
    import time

    import jax
    from jax.sharding import NamedSharding, PartitionSpec

    if "no_collective" not in flags:
        flags = ("no_collective",) + tuple(flags)
    runner = _get_runner(loop_k=n_iters, flags=flags)
    if in_maps is None:
        in_maps = _dummy_in_maps()
    concat_in = _concat_inputs(in_maps, runner["in_names"])
    sharding = NamedSharding(runner["mesh"], PartitionSpec("core"))
    dev_in = [jax.device_put(a, sharding) for a in concat_in]
    for a in dev_in:
        jax.block_until_ready(a)

    best = None
    for r in range(rounds + 1):  # first call = warmup (compile), not timed
        zs = [jax.device_put(z, sharding) for z in _zero_outs(runner)]
        for z in zs:
            jax.block_until_ready(z)
        t0 = time.time()
        out = runner["fn"](*dev_in, *zs)
        jax.block_until_ready(out)
        dt = (time.time() - t0) / n_iters
        if r > 0 and (best is None or dt < best):
            best = dt
    return best


# revision 31
# speedup vs baseline: 2.4697x; 1.1370x over previous
"""QLoRA-style MLP (fake-quant base + LoRA + exact GeLU) on 8 TRN2 cores.

Sharding: token data-parallel (4096 tokens / 8 cores = 512 tokens per core),
weights replicated.  The only cross-core communication is a tiny AllReduce(max)
for the global fake-quant scale of the hidden activation.

Math per layer (matching the jax reference):
    base = fq(x) @ fq(W) + b          fq(t) = clip(round(t/s), -127, 127) * s,
                                      s = max(max|t|, 1e-8) / 127  (global max)
    lora = 2.0 * (x @ A) @ B          (bf16 operands on device)
    out  = base + lora                (layer 1 additionally GeLU'd, exact erf)

v3 design (vs v1): h kept resident in SBUF as fp16 (no HBM spill/reload),
all LoRA matmuls stream bf16 (1 cycle/row on the PE instead of 4 for f32),
inputs packed into 6 DRAM tensors, weights repacked host-side so streaming
DMAs are few and large (16 x 2MB for W_fc, 32 x 1MB for W_proj).

Device mapping (per core, T=512 tokens):
  L1: psum[ff128, T] = sum_ko wfc[mo][k,ff]^T @ qx[k,T]      (bf16 int matmul)
                       + B_fc[16,ff]^T-slice @ xa1[16,T]     (bf16, K=16)
      h[ff,T](fp16) = Gelu(psum * s1 + b_fc); track per-column absmax
  AllReduce(max) -> s_h
  L2: xa2[16,T] = A_proj^T @ h  (64 bf16 matmuls, overlaps the AllReduce)
      qh[ff,T](bf16) = round(h / s_h)   (ACT scale + DVE magic-round)
      psum[tok128, 512] = sum_ko qh[k,tok]^T-tiles @ wproj[no][k,512]
                          + xa2[16,tok]^T-slice @ B_proj[16,512]
      out = psum * s2 + b_proj
"""

import os
import sys

import numpy as np

if "/opt/trn_rl_repo" not in sys.path:
    sys.path.insert(0, "/opt/trn_rl_repo")

import ml_dtypes

# Problem shapes (hardcoded per contract).
B_, S, D, DFF, R = 2, 2048, 2048, 8192, 16
T = B_ * S  # 4096 tokens
NCORES = 8
TC = T // NCORES  # 512 tokens per core
QMAX = np.float32(127.0)
MAGIC = float(np.float32(12582912.0))  # 1.5 * 2**23: fp32 round-half-even trick

KO1 = D // 128  # 16  k-tiles for layer 1
MO1 = DFF // 512  # 16  512-wide ff blocks
M64 = DFF // 128  # 64  128-wide ff blocks
KO2 = DFF // 128  # 64  k-tiles for layer 2
NO2 = D // 512  # 4   512-wide output-col blocks
MT = TC // 128  # 4   token tiles per core
W2CH = 8  # W_proj ko-tiles per streamed chunk
NCH2 = KO2 // W2CH  # 8 chunks per no

_CACHE = {}
LAST_RESULT = None


def _build_nc(n_cores=NCORES, flags=(), loop_k=None):
    """Build + compile the Bass program.

    loop_k: when set, wrap the whole forward pass in a hardware For loop that
    executes it ``loop_k`` times per NEFF launch (used by ``bench`` to measure
    steady-state device throughput without per-dispatch tunnel overhead)."""
    from contextlib import ExitStack

    import concourse.bass as bass  # noqa: F401
    import concourse.mybir as mybir
    import concourse.tile as tile
    from concourse import bacc, bass_isa
    from concourse.bass import ds, ts

    f32 = mybir.dt.float32
    bf16 = mybir.dt.bfloat16
    fp16 = mybir.dt.float16
    AF = mybir.ActivationFunctionType
    ALU = mybir.AluOpType

    nc = bacc.Bacc(None, target_bir_lowering=False, num_devices=n_cores)

    # ---- kernel I/O -------------------------------------------------------
    # blob: all streamed bf16 data in [chunk, 128, 8, 512] granules:
    #   chunks 0-1:  qx^T  (quantized x, int-valued; [128, 16ko, 512])
    #   chunks 2-3:  x^T   (bf16 x for the lora path)
    #   chunks 4-35: W_fc  (chunk 4 + 2*mo + half)
    #   chunks 36-67: W_proj (chunk 36 + 8*no + g)
    blob_t = nc.dram_tensor("blob", [68, 128, 8, 512], bf16, kind="ExternalInput")
    # apk: [:, 0:KO1, :] = A_fc^T-tiles, [:, KO1:KO1+KO2, :] = A_proj^T-tiles
    apk_t = nc.dram_tensor("apk", [128, KO1 + KO2, R], bf16, kind="ExternalInput")
    # bpk: [:, 0:DFF] = B_fc, [:, DFF:DFF+D] = B_proj
    bpk_t = nc.dram_tensor("bpk", [R, DFF + D], bf16, kind="ExternalInput")
    # fpk: [:, 0:M64] biasfc (col mi), [:, M64:M64+D] biasproj, [:, M64+D:] scal
    # scal columns: 0: s1 = s_x*s_wfc, 1: c1 = 2/s1, 2: s_wproj, 3: unused
    fpk_t = nc.dram_tensor("fpk", [128, M64 + D + 4], f32, kind="ExternalInput")
    out_t = nc.dram_tensor("out", [MT, 128, D], f32, kind="ExternalOutput")

    SC = M64 + D  # scal column base in fpk
    p1 = "phase2_only" not in flags
    p2 = "phase1_only" not in flags

    with tile.TileContext(nc) as tc:
        with ExitStack() as ctx:
            consts = ctx.enter_context(tc.tile_pool(name="consts", bufs=1))
            dram = ctx.enter_context(tc.tile_pool(name="dram", bufs=1, space="DRAM"))

            # whole-kernel residents
            fpk_sb = consts.tile([128, M64 + D + 4], f32)
            apk_sb = consts.tile([128, KO1 + KO2, R], bf16)
            bpj_sb = consts.tile([R, D], bf16)
            h_sb = consts.tile([128, KO2, TC], fp16)
            xa2_sb = consts.tile([R, TC], bf16)
            maxcol = consts.tile([128, M64], f32)
            ar_in = dram.tile([128, 1], f32)
            ar_out = dram.tile(
                [128, 1], f32, addr_space="Shared" if n_cores > 4 else "Local"
            )
            nc.scalar.dma_start(fpk_sb[:], fpk_t[:])
            nc.scalar.dma_start(apk_sb[:], apk_t[:])
            nc.scalar.dma_start(bpj_sb[:], bpk_t[:, ds(DFF, D)])
            scal_sb = fpk_sb[:, ds(SC, 4)]

            if loop_k:
                ctx.enter_context(tc.For_i(0, loop_k, 1))

            # ---- phase 1: h = Gelu(s1 * (qx@qW + lora1/s1) + b_fc) -----------
            with tc.tile_pool(name="ph1c", bufs=1) as ph1c, tc.tile_pool(
                name="wfc", bufs=3
            ) as wp, tc.tile_pool(name="ps1", bufs=2, space="PSUM") as pp:
                xpk_sb = ph1c.tile([128, 2 * KO1, TC], bf16)
                bfc_sb = ph1c.tile([R, DFF], bf16)
                xa1_sb = ph1c.tile([R, TC], bf16)
                if p1:
                    # sync queue: qx chunks interleaved with the first weight
                    # halves so the first matmul isn't stuck behind the whole x
                    # pack; scalar queue: lora consts + unquantized x.
                    w_mo0 = wp.tile([128, KO1, 512], bf16, tag="wfc", name="w_mo")
                    nc.sync.dma_start(xpk_sb[:, ds(0, 4), :], blob_t[0, :, ds(0, 4), :])
                    nc.sync.dma_start(w_mo0[:, ds(0, 8), :], blob_t[4])
                    nc.sync.dma_start(xpk_sb[:, ds(4, 4), :], blob_t[0, :, ds(4, 4), :])
                    nc.sync.dma_start(w_mo0[:, ds(8, 8), :], blob_t[5])
                    nc.sync.dma_start(xpk_sb[:, ds(8, 8), :], blob_t[1])
                    nc.scalar.dma_start(bfc_sb[:], bpk_t[:, ds(0, DFF)])
                    for c in range(2):
                        nc.scalar.dma_start(
                            xpk_sb[:, ds(KO1 + 8 * c, 8), :], blob_t[2 + c]
                        )

                for mo in range(MO1 if p1 else 0):
                    if mo == 0:
                        w_mo = w_mo0
                    else:
                        w_mo = wp.tile([128, KO1, 512], bf16, tag="wfc", name="w_mo")
                        for half in range(2):
                            nc.sync.dma_start(
                                w_mo[:, ds(8 * half, 8), :],
                                blob_t[4 + 2 * mo + half],
                            )
                    pss = [
                        pp.tile([128, TC], f32, tag=f"ps1_{i}", name="ps1t")
                        for i in range(4)
                    ]
                    for ko in range(KO1):
                        for sub in range(4):
                            nc.tensor.matmul(
                                pss[sub][:],
                                w_mo[:, ko, ts(sub, 128)],
                                xpk_sb[:, ko, :],
                                start=(ko == 0),
                                stop=False,
                            )
                    if mo == 0:
                        # xa1 = bf16((x @ A_fc)^T * (2/s1))
                        ps_a1 = pp.tile([128, TC], f32, tag="ps1_0", name="psa1")
                        for ko in range(KO1):
                            nc.tensor.matmul(
                                ps_a1[:R, :],
                                apk_sb[:, ko, :],
                                xpk_sb[:, KO1 + ko, :],
                                start=(ko == 0),
                                stop=(ko == KO1 - 1),
                            )
                        nc.vector.tensor_scalar_mul(
                            xa1_sb[:], ps_a1[:R, :], scal_sb[:R, 1:2]
                        )
                    for sub in range(4):
                        mi = 4 * mo + sub
                        nc.tensor.matmul(
                            pss[sub][:],
                            bfc_sb[:, ts(mi, 128)],
                            xa1_sb[:],
                            start=False,
                            stop=True,
                        )
                        nc.scalar.activation(
                            h_sb[:, mi, :],
                            pss[sub][:],
                            AF.Gelu,
                            bias=fpk_sb[:, mi : mi + 1],
                            scale=scal_sb[:, 0:1],
                        )
                        nc.vector.tensor_reduce(
                            maxcol[:, mi : mi + 1],
                            h_sb[:, mi, :],
                            axis=mybir.AxisListType.X,
                            op=ALU.max,
                            apply_absolute_value=True,
                        )

            # ---- phase 1.5: global scale via AllReduce(max) ------------------
            if p2:
                if not p1:
                    nc.vector.memset(maxcol[:], 1.0)
                    nc.vector.memset(h_sb[:], 0)
                pmax = consts.tile([128, 1], f32)
                nc.vector.tensor_reduce(
                    pmax[:], maxcol[:], axis=mybir.AxisListType.X, op=ALU.max
                )
                armax = consts.tile([128, 1], f32)
                if "no_collective" in flags:
                    nc.vector.tensor_copy(armax[:], pmax[:])
                else:
                    nc.gpsimd.dma_start(ar_in[:], pmax[:])
                    nc.gpsimd.collective_compute(
                        "AllReduce",
                        ALU.max,
                        replica_groups=[list(range(n_cores))],
                        ins=[ar_in[:]],
                        outs=[ar_out[:]],
                    )
                    nc.gpsimd.dma_start(armax[:], ar_out[:])
                gmax = consts.tile([128, 1], f32)
                if "no_par_reduce" in flags:
                    nc.vector.tensor_copy(gmax[:], armax[:])
                else:
                    nc.gpsimd.partition_all_reduce(
                        gmax[:], armax[:], channels=128, reduce_op=bass_isa.ReduceOp.max
                    )
                scaleh = consts.tile([128, 1], f32)
                invsh = consts.tile([128, 1], f32)
                s2v = consts.tile([128, 1], f32)
                c2v = consts.tile([128, 1], f32)
                nc.vector.tensor_scalar_max(gmax[:], gmax[:], 1e-8)
                # scale_h = gmax / 127  (mult by fp32(1/127): <=1ulp from divide)
                nc.vector.tensor_scalar_mul(
                    scaleh[:], gmax[:], float(np.float32(1.0) / np.float32(127.0))
                )
                nc.vector.reciprocal(invsh[:], scaleh[:])
                nc.vector.tensor_tensor(s2v[:], scaleh[:], scal_sb[:, 2:3], op=ALU.mult)
                nc.vector.reciprocal(c2v[:], s2v[:])
                nc.vector.tensor_scalar_mul(c2v[:], c2v[:], 2.0)

            # ---- phase 2: out = s2 * (qh@qW2 + lora2/s2) + b_proj ------------
            # xa2's 64 s_h-independent matmuls cross the AllReduce barrier on
            # the PE while qh production (ACT+DVE) waits for s_h.
            if not p2:
                pass
            else:
             with tc.tile_pool(name="qh", bufs=1) as qhp, tc.tile_pool(
                name="w2", bufs=3
            ) as w2p, tc.tile_pool(name="qt", bufs=3) as qtp, tc.tile_pool(
                name="ps2", bufs=2, space="PSUM"
            ) as pp2, tc.tile_pool(name="ot", bufs=4) as otp:
                qh_sb = qhp.tile([128, KO2, TC], bf16)
                ps_a2 = pp2.tile([128, TC], f32, tag="ps2_0", name="psa2")
                w2_first = []
                for ko in range(KO2):
                    if ko % W2CH == 0:
                        w2_sb = w2p.tile([128, W2CH, 512], bf16, tag="w2", name="w2c")
                        nc.sync.dma_start(w2_sb[:], blob_t[36 + ko // W2CH])
                        w2_first.append(w2_sb)
                    nc.tensor.matmul(
                        ps_a2[:R, :],
                        apk_sb[:, KO1 + ko, :],
                        h_sb[:, ko, :],
                        start=(ko == 0),
                        stop=(ko == KO2 - 1),
                    )
                    qt = qtp.tile([128, TC], f32, tag="qt", name="qt")
                    nc.scalar.activation(
                        qt[:], h_sb[:, ko, :], AF.Copy, bias=0.0, scale=invsh[:, 0:1]
                    )
                    nc.vector.tensor_scalar(
                        qh_sb[:, ko, :],
                        qt[:],
                        MAGIC,
                        MAGIC,
                        op0=ALU.add,
                        op1=ALU.subtract,
                    )
                nc.vector.tensor_scalar_mul(xa2_sb[:], ps_a2[:R, :], c2v[:R, 0:1])

                for no in range(NO2):
                    ps_list = [
                        pp2.tile([128, 512], f32, tag=f"ps2_{mi}", name="ps2t")
                        for mi in range(MT)
                    ]
                    for ko in range(KO2):
                        ch = ko // W2CH
                        if no == 0:
                            w2_sb = w2_first[ch]
                        elif ko % W2CH == 0:
                            w2_sb = w2p.tile(
                                [128, W2CH, 512], bf16, tag="w2", name="w2c"
                            )
                            nc.sync.dma_start(
                                w2_sb[:], blob_t[36 + 8 * no + ko // W2CH]
                            )
                        for mi in range(MT):
                            nc.tensor.matmul(
                                ps_list[mi][:],
                                qh_sb[:, ko, ts(mi, 128)],
                                w2_sb[:, ko % W2CH, :],
                                start=(ko == 0),
                                stop=False,
                            )
                    for mi in range(MT):
                        nc.tensor.matmul(
                            ps_list[mi][:],
                            xa2_sb[:, ts(mi, 128)],
                            bpj_sb[:, ds(no * 512, 512)],
                            start=False,
                            stop=True,
                        )
                        ot = otp.tile([128, 512], f32, tag="ot", name="ot")
                        # scale on ACT, bias-add on DVE (halves eviction latency
                        # at psum-bank reuse boundaries)
                        nc.scalar.activation(
                            ot[:], ps_list[mi][:], AF.Copy, bias=0.0, scale=s2v[:, 0:1]
                        )
                        nc.vector.tensor_add(
                            ot[:], ot[:], fpk_sb[:, ds(M64 + no * 512, 512)]
                        )
                        nc.scalar.dma_start(out_t[mi, :, ds(no * 512, 512)], ot[:])

    nc.compile()
    return nc


def _scale_of(a):
    m = np.max(np.abs(a)).astype(np.float32)
    m = np.maximum(m, np.float32(1e-8))
    return (m / QMAX).astype(np.float32)


def _quant(a, s):
    return np.clip(np.round(a / s), -QMAX, QMAX)


def _prep_weights(W_fc, b_fc, A_fc, B_fc, W_proj, b_proj, A_proj, B_proj):
    """Shared (replicated) input tensors from the weight arrays."""
    bf16 = ml_dtypes.bfloat16
    W_fc = np.asarray(W_fc, np.float32)
    W_proj = np.asarray(W_proj, np.float32)

    s_wfc = _scale_of(W_fc)
    s_wp = _scale_of(W_proj)
    qwfc = _quant(W_fc, s_wfc)
    qwp = _quant(W_proj, s_wp)

    # [k, ff] -> [mo, p(k%128), ko, c(ff%512)] -> chunks [2mo+half, p, 8, 512]
    wfc_dev = (
        qwfc.reshape(KO1, 128, MO1, 512)
        .transpose(2, 1, 0, 3)
        .astype(bf16)
        .reshape(MO1, 128, 2, W2CH, 512)
        .transpose(0, 2, 1, 3, 4)
        .reshape(2 * MO1, 128, W2CH, 512)
    )
    # [k, d] -> [no, p(k%128), ko, c(d%512)] -> chunks [8no+g, p, 8, 512]
    wpj_dev = (
        qwp.reshape(KO2, 128, NO2, 512)
        .transpose(2, 1, 0, 3)
        .astype(bf16)
        .reshape(NO2, 128, NCH2, W2CH, 512)
        .transpose(0, 2, 1, 3, 4)
        .reshape(NO2 * NCH2, 128, W2CH, 512)
    )
    wchunks = np.ascontiguousarray(
        np.concatenate([wfc_dev, wpj_dev], axis=0)
    )  # [64, 128, 8, 512]
    afc = np.asarray(A_fc, np.float32).reshape(KO1, 128, R).transpose(1, 0, 2)
    apj = np.asarray(A_proj, np.float32).reshape(KO2, 128, R).transpose(1, 0, 2)
    apk_dev = np.ascontiguousarray(
        np.concatenate([afc, apj], axis=1).astype(bf16)
    )
    bpk_dev = np.ascontiguousarray(
        np.concatenate(
            [np.asarray(B_fc, np.float32), np.asarray(B_proj, np.float32)], axis=1
        ).astype(bf16)
    )
    fpk_dev = np.empty((128, M64 + D + 4), np.float32)
    fpk_dev[:, 0:M64] = np.asarray(b_fc, np.float32).reshape(M64, 128).T
    fpk_dev[:, M64 : M64 + D] = np.asarray(b_proj, np.float32)[None, :]
    return {
        "wchunks": wchunks,
        "apk": apk_dev,
        "bpk": bpk_dev,
        "fpk": fpk_dev,
        "s_wfc": s_wfc,
        "s_wp": s_wp,
    }


def _prep_inputs(hidden_states, W_fc, b_fc, A_fc, B_fc, W_proj, b_proj, A_proj, B_proj):
    bf16 = ml_dtypes.bfloat16
    w = _prep_weights(W_fc, b_fc, A_fc, B_fc, W_proj, b_proj, A_proj, B_proj)
    x = np.ascontiguousarray(np.asarray(hidden_states, np.float32).reshape(T, D))
    s_x = _scale_of(x)
    qx = _quant(x, s_x)
    s1 = np.float32(s_x * w["s_wfc"])
    c1 = np.float32(np.float32(2.0) / s1)
    fpk = w["fpk"].copy()
    fpk[:, M64 + D :] = np.array([s1, c1, w["s_wp"], 0.0], np.float32)[None, :]

    shared = {"apk": w["apk"], "bpk": w["bpk"], "fpk": fpk}
    in_maps = []
    for c in range(NCORES):
        xc = x[c * TC : (c + 1) * TC]  # [TC, D]
        qxc = qx[c * TC : (c + 1) * TC]
        qxT = qxc.T.reshape(KO1, 128, TC).transpose(1, 0, 2)
        xT = xc.T.reshape(KO1, 128, TC).transpose(1, 0, 2)
        # [128, 32ko, 512] -> 4 chunks [c, 128, 8, 512], then weight chunks
        xpk = (
            np.concatenate([qxT, xT], axis=1)
            .astype(bf16)
            .reshape(128, 4, W2CH, TC)
            .transpose(1, 0, 2, 3)
        )
        blob = np.ascontiguousarray(np.concatenate([xpk, w["wchunks"]], axis=0))
        in_maps.append({**shared, "blob": blob})
    return in_maps


def _get_runner(**build_kwargs):
    """Build the Bass program once and wrap it in a cached jitted shard_map
    executable (adapted from concourse.bass2jax.run_bass_via_pjrt)."""
    key = ("runner", tuple(sorted(build_kwargs.items())))
    if key in _CACHE:
        return _CACHE[key]
    nc = _build_nc(**build_kwargs)
    n_cores_ = build_kwargs.get("n_cores", NCORES)
    runner = _runner_from_nc(nc, n_cores_)
    _CACHE[key] = runner
    return runner


def _runner_from_nc(nc, n_cores_):
    import jax
    import jax.numpy as jnp  # noqa: F401
    from jax.experimental.shard_map import shard_map
    from jax.sharding import Mesh, PartitionSpec

    from concourse import bass2jax, mybir

    bass2jax.install_neuronx_cc_hook()
    assert nc.dbg_addr is None
    partition_name = nc.partition_id_tensor.name if nc.partition_id_tensor else None

    in_names = []
    out_names = []
    out_avals = []
    for alloc in nc.m.functions[0].allocations:
        if not isinstance(alloc, mybir.MemoryLocationSet):
            continue
        name = alloc.memorylocations[0].name
        if alloc.kind == "ExternalInput":
            if name != partition_name:
                in_names.append(name)
        elif alloc.kind == "ExternalOutput":
            out_names.append(name)
            out_avals.append(
                jax.core.ShapedArray(tuple(alloc.tensor_shape), mybir.dt.np(alloc.dtype))
            )
    all_in_names = tuple(in_names) + tuple(out_names)
    if partition_name is not None:
        all_in_names = all_in_names + (partition_name,)
    n_params = len(in_names)
    n_outs = len(out_names)

    def _body(*args):
        operands = list(args)
        if partition_name is not None:
            operands.append(bass2jax.partition_id_tensor())
        outs = bass2jax._bass_exec_p.bind(
            *operands,
            out_avals=tuple(out_avals),
            in_names=all_in_names,
            out_names=tuple(out_names),
            lowering_input_output_aliases=(),
            sim_require_finite=True,
            sim_require_nnan=True,
            nc=nc,
        )
        return tuple(outs)

    devices = jax.devices()[:n_cores_]
    assert len(devices) == n_cores_, f"need {n_cores_} devices, have {len(jax.devices())}"
    mesh = Mesh(np.asarray(devices), ("core",))
    in_specs = (PartitionSpec("core"),) * (n_params + n_outs)
    out_specs = (PartitionSpec("core"),) * n_outs
    donate = tuple(range(n_params, n_params + n_outs))
    fn = jax.jit(
        shard_map(
            _body, mesh=mesh, in_specs=in_specs, out_specs=out_specs, check_rep=False
        ),
        donate_argnums=donate,
        keep_unused=True,
    )
    runner = {
        "fn": fn,
        "in_names": in_names,
        "out_names": out_names,
        "out_avals": out_avals,
        "mesh": mesh,
        "nc": nc,
    }
    runner["n_cores"] = n_cores_
    return runner


def _zero_outs(runner):
    n = runner["n_cores"]
    return [
        np.zeros((n * a.shape[0], *a.shape[1:]), a.dtype) for a in runner["out_avals"]
    ]


def _concat_inputs(in_maps, in_names):
    return [
        np.concatenate([m[name] for m in in_maps], axis=0) for name in in_names
    ]


def kernel(hidden_states, W_fc, b_fc, A_fc, B_fc, W_proj, b_proj, A_proj, B_proj):
    global LAST_RESULT
    runner = _get_runner()
    in_maps = _prep_inputs(
        hidden_states, W_fc, b_fc, A_fc, B_fc, W_proj, b_proj, A_proj, B_proj
    )
    concat_in = _concat_inputs(in_maps, runner["in_names"])
    out_arrs = runner["fn"](*concat_in, *_zero_outs(runner))
    (out_global,) = [np.asarray(a) for a in out_arrs]
    # out_global: [NCORES*MT, 128, D] -> per-core [MT,128,D] -> tokens x D
    out = out_global.reshape(T, D).astype(np.float32)
    return out.reshape(B_, S, D)


def _dummy_in_maps():
    rng = np.random.default_rng(0)
    dummy = {
        "hidden_states": rng.standard_normal((B_, S, D), dtype=np.float32),
        "W_fc": rng.standard_normal((D, DFF), dtype=np.float32) / 45.0,
        "b_fc": np.zeros(DFF, np.float32),
        "A_fc": rng.standard_normal((D, R), dtype=np.float32) / 45.0,
        "B_fc": rng.standard_normal((R, DFF), dtype=np.float32) * 0.01,
        "W_proj": rng.standard_normal((DFF, D), dtype=np.float32) / 90.0,
        "b_proj": np.zeros(D, np.float32),
        "A_proj": rng.standard_normal((DFF, R), dtype=np.float32) / 90.0,
        "B_proj": rng.standard_normal((R, D), dtype=np.float32) * 0.01,
    }
    return _prep_inputs(**dummy)


def bench(n_iters=20, in_maps=None, rounds=5, **build_kwargs):
    """Per-iteration wall time of the full kernel (one dispatch per forward
    pass, device-resident inputs), best of ``rounds`` runs of ``n_iters``
    iterations each.  The axon tunnel's dispatch cost degrades one-sidedly
    over a session, so min-of-rounds is the robust steady-state estimate."""
    import time

    import jax
    from jax.sharding import NamedSharding, PartitionSpec

    runner = _get_runner(**build_kwargs)
    if in_maps is None:
        in_maps = _dummy_in_maps()
    concat_in = _concat_inputs(in_maps, runner["in_names"])
    sharding = NamedSharding(runner["mesh"], PartitionSpec("core"))
    dev_in = [jax.device_put(a, sharding) for a in concat_in]
    for a in dev_in:
        jax.block_until_ready(a)

    best = None
    for r in range(rounds):
        zero_sets = [
            [jax.device_put(z, sharding) for z in _zero_outs(runner)]
            for _ in range(n_iters + 1)
        ]
        for zs in zero_sets:
            for z in zs:
                jax.block_until_ready(z)
        out = runner["fn"](*dev_in, *zero_sets[-1])  # warmup (compile on r=0)
        jax.block_until_ready(out)
        t0 = time.time()
        for i in range(n_iters):
            out = runner["fn"](*dev_in, *zero_sets[i])
        jax.block_until_ready(out)
        dt = (time.time() - t0) / n_iters
        if best is None or dt < best:
            best = dt
    return best


def bench_device_loop(n_iters=20, in_maps=None, rounds=2, flags=()):
    """Device-side steady-state throughput: one dispatch executes a NEFF whose
    body is a hardware For loop running the forward pass ``n_iters`` times
    back-to-back, amortizing the per-call tunnel dispatch (~1.7ms on this
    setup).  The cross-core AllReduce desyncs the collective fabric when
    executed inside a hardware loop here, so this diagnostic runs the
    no_collective build (identical kernel minus the inter-core max exchange)."""
    import time

    import jax
    from jax.sharding import NamedSharding, PartitionSpec

    if "no_collective" not in flags:
        flags = ("no_collective",) + tuple(flags)
    runner = _get_runner(loop_k=n_iters, flags=flags)
    if in_maps is None:
        in_maps = _dummy_in_maps()
    concat_in = _concat_inputs(in_maps, runner["in_names"])
    sharding = NamedSharding(runner["mesh"], PartitionSpec("core"))
    dev_in = [jax.device_put(a, sharding) for a in concat_in]
    for a in dev_in:
        jax.block_until_ready(a)

    best = None
    for r in range(rounds + 1):  # first call = warmup (compile), not timed
        zs = [jax.device_put(z, sharding) for z in _zero_outs(runner)]
        for z in zs:
            jax.block_until_ready(z)
        t0 = time.time()
        out = runner["fn"](*dev_in, *zs)
        jax.block_until_ready(out)
        dt = (time.time() - t0) / n_iters
        if r > 0 and (best is None or dt < best):
            best = dt
    return best


# revision 39
# speedup vs baseline: 2.7475x; 1.1125x over previous
"""QLoRA-style MLP (fake-quant base + LoRA + exact GeLU) on 8 TRN2 cores.

Sharding: token data-parallel (4096 tokens / 8 cores = 512 tokens per core),
weights replicated.  The only cross-core communication is a tiny AllReduce(max)
for the global fake-quant scale of the hidden activation.

Math per layer (matching the jax reference):
    base = fq(x) @ fq(W) + b          fq(t) = clip(round(t/s), -127, 127) * s,
                                      s = max(max|t|, 1e-8) / 127  (global max)
    lora = 2.0 * (x @ A) @ B          (bf16 operands on device)
    out  = base + lora                (layer 1 additionally GeLU'd, exact erf)

v3 design (vs v1): h kept resident in SBUF as fp16 (no HBM spill/reload),
all LoRA matmuls stream bf16 (1 cycle/row on the PE instead of 4 for f32),
inputs packed into 6 DRAM tensors, weights repacked host-side so streaming
DMAs are few and large (16 x 2MB for W_fc, 32 x 1MB for W_proj).

Device mapping (per core, T=512 tokens):
  L1: psum[ff128, T] = sum_ko wfc[mo][k,ff]^T @ qx[k,T]      (bf16 int matmul)
                       + B_fc[16,ff]^T-slice @ xa1[16,T]     (bf16, K=16)
      h[ff,T](fp16) = Gelu(psum * s1 + b_fc); track per-column absmax
  AllReduce(max) -> s_h
  L2: xa2[16,T] = A_proj^T @ h  (64 bf16 matmuls, overlaps the AllReduce)
      qh[ff,T](bf16) = round(h / s_h)   (ACT scale + DVE magic-round)
      psum[tok128, 512] = sum_ko qh[k,tok]^T-tiles @ wproj[no][k,512]
                          + xa2[16,tok]^T-slice @ B_proj[16,512]
      out = psum * s2 + b_proj
"""

import os
import sys

import numpy as np

if "/opt/trn_rl_repo" not in sys.path:
    sys.path.insert(0, "/opt/trn_rl_repo")

import ml_dtypes

# Problem shapes (hardcoded per contract).
B_, S, D, DFF, R = 2, 2048, 2048, 8192, 16
T = B_ * S  # 4096 tokens
NCORES = 8
TC = T // NCORES  # 512 tokens per core
QMAX = np.float32(127.0)
MAGIC = float(np.float32(12582912.0))  # 1.5 * 2**23: fp32 round-half-even trick

KO1 = D // 128  # 16  k-tiles for layer 1
MO1 = DFF // 512  # 16  512-wide ff blocks
M64 = DFF // 128  # 64  128-wide ff blocks
KO2 = DFF // 128  # 64  k-tiles for layer 2
NO2 = D // 512  # 4   512-wide output-col blocks
MT = TC // 128  # 4   token tiles per core
W2CH = 8  # W_proj ko-tiles per streamed chunk
NCH2 = KO2 // W2CH  # 8 chunks per no

_CACHE = {}
LAST_RESULT = None


def _build_nc(n_cores=NCORES, flags=(), loop_k=None):
    """Build + compile the Bass program.

    loop_k: when set, wrap the whole forward pass in a hardware For loop that
    executes it ``loop_k`` times per NEFF launch (used by ``bench`` to measure
    steady-state device throughput without per-dispatch tunnel overhead)."""
    from contextlib import ExitStack

    import concourse.bass as bass  # noqa: F401
    import concourse.mybir as mybir
    import concourse.tile as tile
    from concourse import bacc, bass_isa
    from concourse.bass import ds, ts

    f32 = mybir.dt.float32
    bf16 = mybir.dt.bfloat16
    fp16 = mybir.dt.float16
    AF = mybir.ActivationFunctionType
    ALU = mybir.AluOpType

    nc = bacc.Bacc(None, target_bir_lowering=False, num_devices=n_cores)

    # ---- kernel I/O -------------------------------------------------------
    # blob: all bf16 data in [chunk, 128, 8, 512] granules:
    #   chunks 0-1:  qx^T  (quantized x, int-valued; [128, 16ko, 512])
    #   chunks 2-3:  x^T   (bf16 x for the lora path)
    #   chunks 4-35: W_fc  (chunk 4 + 2*mo + half)
    #   chunks 36-67: W_proj (chunk 36 + 8*no + g)
    #   chunk 68:    A_fc^T ++ A_proj^T tiles ([128, 80, 16], padded)
    #   chunks 69-70: B_fc   ([16, 8192], partitions 0-15 only)
    #   chunk 71:    B_proj ([16, 2048], partitions 0-15 only)
    blob_t = nc.dram_tensor("blob", [72, 128, 8, 512], bf16, kind="ExternalInput")
    # fpk: [:, 0:M64] biasfc (col mi), [:, M64:M64+D] biasproj, [:, M64+D:] scal
    # scal columns: 0: s1 = s_x*s_wfc, 1: c1 = 2/s1, 2: s_wproj, 3: unused
    fpk_t = nc.dram_tensor("fpk", [128, M64 + D + 4], f32, kind="ExternalInput")
    out_t = nc.dram_tensor("out", [MT, 128, D], f32, kind="ExternalOutput")

    SC = M64 + D  # scal column base in fpk
    p1 = "phase2_only" not in flags
    p2 = "phase1_only" not in flags

    with tile.TileContext(nc) as tc:
        with ExitStack() as ctx:
            consts = ctx.enter_context(tc.tile_pool(name="consts", bufs=1))
            dram = ctx.enter_context(tc.tile_pool(name="dram", bufs=1, space="DRAM"))

            # whole-kernel residents
            fpk_sb = consts.tile([128, M64 + D + 4], f32)
            apk_sb = consts.tile([128, KO1 + KO2, R], bf16)
            bpj_sb = consts.tile([R, D], bf16)
            h_sb = consts.tile([128, KO2, TC], fp16)
            xa2_sb = consts.tile([R, TC], bf16)
            maxcol = consts.tile([128, M64], f32)
            ar_in = dram.tile([128, 1], f32)
            ar_out = dram.tile(
                [128, 1], f32, addr_space="Shared" if n_cores > 4 else "Local"
            )
            nc.scalar.dma_start(fpk_sb[:], fpk_t[:])
            nc.scalar.dma_start(apk_sb[:, ds(0, 64), :], blob_t[68, :, ds(0, 2), :])
            nc.scalar.dma_start(apk_sb[:, ds(64, 16), :], blob_t[68, :, 2, ds(0, 256)])
            nc.scalar.dma_start(bpj_sb[:], blob_t[71, ds(0, R), ds(0, 4), :])
            scal_sb = fpk_sb[:, ds(SC, 4)]

            if loop_k:
                ctx.enter_context(tc.For_i(0, loop_k, 1))

            # ---- phase 1: h = Gelu(s1 * (qx@qW + lora1/s1) + b_fc) -----------
            with tc.tile_pool(name="ph1c", bufs=1) as ph1c, tc.tile_pool(
                name="wfc", bufs=3
            ) as wp, tc.tile_pool(name="ps1", bufs=2, space="PSUM") as pp:
                xpk_sb = ph1c.tile([128, 2 * KO1, TC], bf16)
                bfc_sb = ph1c.tile([R, DFF], bf16)
                xa1_sb = ph1c.tile([R, TC], bf16)
                if p1:
                    # sync queue: qx chunks interleaved with the first weight
                    # halves so the first matmul isn't stuck behind the whole x
                    # pack; scalar queue: lora consts + unquantized x.
                    w_mo0 = wp.tile([128, KO1, 512], bf16, tag="wfc", name="w_mo")
                    nc.sync.dma_start(xpk_sb[:, ds(0, 4), :], blob_t[0, :, ds(0, 4), :])
                    nc.sync.dma_start(w_mo0[:, ds(0, 8), :], blob_t[4])
                    nc.sync.dma_start(xpk_sb[:, ds(4, 4), :], blob_t[0, :, ds(4, 4), :])
                    nc.sync.dma_start(w_mo0[:, ds(8, 8), :], blob_t[5])
                    nc.sync.dma_start(xpk_sb[:, ds(8, 8), :], blob_t[1])
                    nc.scalar.dma_start(bfc_sb[:, ds(0, 4096)], blob_t[69, ds(0, R)])
                    nc.scalar.dma_start(bfc_sb[:, ds(4096, 4096)], blob_t[70, ds(0, R)])
                    for c in range(2):
                        nc.scalar.dma_start(
                            xpk_sb[:, ds(KO1 + 8 * c, 8), :], blob_t[2 + c]
                        )

                for mo in range(MO1 if p1 else 0):
                    if mo == 0:
                        w_mo = w_mo0
                    else:
                        w_mo = wp.tile([128, KO1, 512], bf16, tag="wfc", name="w_mo")
                        for half in range(2):
                            nc.sync.dma_start(
                                w_mo[:, ds(8 * half, 8), :],
                                blob_t[4 + 2 * mo + half],
                            )
                    pss = [
                        pp.tile([128, TC], f32, tag=f"ps1_{i}", name="ps1t")
                        for i in range(4)
                    ]
                    for ko in range(KO1):
                        for sub in range(4):
                            nc.tensor.matmul(
                                pss[sub][:],
                                w_mo[:, ko, ts(sub, 128)],
                                xpk_sb[:, ko, :],
                                start=(ko == 0),
                                stop=False,
                            )
                    if mo == 0:
                        # xa1 = bf16((x @ A_fc)^T * (2/s1))
                        ps_a1 = pp.tile([128, TC], f32, tag="ps1_0", name="psa1")
                        for ko in range(KO1):
                            nc.tensor.matmul(
                                ps_a1[:R, :],
                                apk_sb[:, ko, :],
                                xpk_sb[:, KO1 + ko, :],
                                start=(ko == 0),
                                stop=(ko == KO1 - 1),
                            )
                        nc.vector.tensor_scalar_mul(
                            xa1_sb[:], ps_a1[:R, :], scal_sb[:R, 1:2]
                        )
                    for sub in range(4):
                        mi = 4 * mo + sub
                        nc.tensor.matmul(
                            pss[sub][:],
                            bfc_sb[:, ts(mi, 128)],
                            xa1_sb[:],
                            start=False,
                            stop=True,
                        )
                        nc.scalar.activation(
                            h_sb[:, mi, :],
                            pss[sub][:],
                            AF.Gelu,
                            bias=fpk_sb[:, mi : mi + 1],
                            scale=scal_sb[:, 0:1],
                        )
                        nc.vector.tensor_reduce(
                            maxcol[:, mi : mi + 1],
                            h_sb[:, mi, :],
                            axis=mybir.AxisListType.X,
                            op=ALU.max,
                            apply_absolute_value=True,
                        )

            # ---- phase 1.5: global scale via AllReduce(max) ------------------
            if p2:
                if not p1:
                    nc.vector.memset(maxcol[:], 1.0)
                    nc.vector.memset(h_sb[:], 0)
                pmax = consts.tile([128, 1], f32)
                nc.vector.tensor_reduce(
                    pmax[:], maxcol[:], axis=mybir.AxisListType.X, op=ALU.max
                )
                armax = consts.tile([128, 1], f32)
                if "no_collective" in flags:
                    nc.vector.tensor_copy(armax[:], pmax[:])
                else:
                    nc.gpsimd.dma_start(ar_in[:], pmax[:])
                    nc.gpsimd.collective_compute(
                        "AllReduce",
                        ALU.max,
                        replica_groups=[list(range(n_cores))],
                        ins=[ar_in[:]],
                        outs=[ar_out[:]],
                    )
                    nc.gpsimd.dma_start(armax[:], ar_out[:])
                gmax = consts.tile([128, 1], f32)
                if "no_par_reduce" in flags:
                    nc.vector.tensor_copy(gmax[:], armax[:])
                else:
                    nc.gpsimd.partition_all_reduce(
                        gmax[:], armax[:], channels=128, reduce_op=bass_isa.ReduceOp.max
                    )
                scaleh = consts.tile([128, 1], f32)
                invsh = consts.tile([128, 1], f32)
                s2v = consts.tile([128, 1], f32)
                c2v = consts.tile([128, 1], f32)
                nc.vector.tensor_scalar_max(gmax[:], gmax[:], 1e-8)
                # scale_h = gmax / 127  (mult by fp32(1/127): <=1ulp from divide)
                nc.vector.tensor_scalar_mul(
                    scaleh[:], gmax[:], float(np.float32(1.0) / np.float32(127.0))
                )
                nc.vector.reciprocal(invsh[:], scaleh[:])
                nc.vector.tensor_tensor(s2v[:], scaleh[:], scal_sb[:, 2:3], op=ALU.mult)
                nc.vector.reciprocal(c2v[:], s2v[:])
                nc.vector.tensor_scalar_mul(c2v[:], c2v[:], 2.0)

            # ---- phase 2: out = s2 * (qh@qW2 + lora2/s2) + b_proj ------------
            # xa2's 64 s_h-independent matmuls cross the AllReduce barrier on
            # the PE while qh production (ACT+DVE) waits for s_h.
            if not p2:
                pass
            else:
             with tc.tile_pool(name="qh", bufs=1) as qhp, tc.tile_pool(
                name="w2", bufs=3
            ) as w2p, tc.tile_pool(name="qt", bufs=3) as qtp, tc.tile_pool(
                name="ps2", bufs=2, space="PSUM"
            ) as pp2, tc.tile_pool(name="ot", bufs=4) as otp:
                qh_sb = qhp.tile([128, KO2, TC], bf16)
                ps_a2 = pp2.tile([128, TC], f32, tag="ps2_0", name="psa2")
                w2_first = []
                for ko in range(KO2):
                    if ko % W2CH == 0:
                        w2_sb = w2p.tile([128, W2CH, 512], bf16, tag="w2", name="w2c")
                        nc.sync.dma_start(w2_sb[:], blob_t[36 + ko // W2CH])
                        w2_first.append(w2_sb)
                    nc.tensor.matmul(
                        ps_a2[:R, :],
                        apk_sb[:, KO1 + ko, :],
                        h_sb[:, ko, :],
                        start=(ko == 0),
                        stop=(ko == KO2 - 1),
                    )
                    qt = qtp.tile([128, TC], f32, tag="qt", name="qt")
                    nc.scalar.activation(
                        qt[:], h_sb[:, ko, :], AF.Copy, bias=0.0, scale=invsh[:, 0:1]
                    )
                    nc.vector.tensor_scalar(
                        qh_sb[:, ko, :],
                        qt[:],
                        MAGIC,
                        MAGIC,
                        op0=ALU.add,
                        op1=ALU.subtract,
                    )
                nc.vector.tensor_scalar_mul(xa2_sb[:], ps_a2[:R, :], c2v[:R, 0:1])

                for no in range(NO2):
                    ps_list = [
                        pp2.tile([128, 512], f32, tag=f"ps2_{mi}", name="ps2t")
                        for mi in range(MT)
                    ]
                    for ko in range(KO2):
                        ch = ko // W2CH
                        if no == 0:
                            w2_sb = w2_first[ch]
                        elif ko % W2CH == 0:
                            w2_sb = w2p.tile(
                                [128, W2CH, 512], bf16, tag="w2", name="w2c"
                            )
                            nc.sync.dma_start(
                                w2_sb[:], blob_t[36 + 8 * no + ko // W2CH]
                            )
                        for mi in range(MT):
                            nc.tensor.matmul(
                                ps_list[mi][:],
                                qh_sb[:, ko, ts(mi, 128)],
                                w2_sb[:, ko % W2CH, :],
                                start=(ko == 0),
                                stop=False,
                            )
                    for mi in range(MT):
                        nc.tensor.matmul(
                            ps_list[mi][:],
                            xa2_sb[:, ts(mi, 128)],
                            bpj_sb[:, ds(no * 512, 512)],
                            start=False,
                            stop=True,
                        )
                        ot = otp.tile([128, 512], f32, tag="ot", name="ot")
                        # scale on ACT, bias-add on DVE (halves eviction latency
                        # at psum-bank reuse boundaries)
                        nc.scalar.activation(
                            ot[:], ps_list[mi][:], AF.Copy, bias=0.0, scale=s2v[:, 0:1]
                        )
                        nc.vector.tensor_add(
                            ot[:], ot[:], fpk_sb[:, ds(M64 + no * 512, 512)]
                        )
                        nc.scalar.dma_start(out_t[mi, :, ds(no * 512, 512)], ot[:])

    nc.compile()
    return nc


def _scale_of(a):
    m = np.max(np.abs(a)).astype(np.float32)
    m = np.maximum(m, np.float32(1e-8))
    return (m / QMAX).astype(np.float32)


def _quant(a, s):
    return np.clip(np.round(a / s), -QMAX, QMAX)


def _prep_weights(W_fc, b_fc, A_fc, B_fc, W_proj, b_proj, A_proj, B_proj):
    """Shared (replicated) input tensors from the weight arrays."""
    bf16 = ml_dtypes.bfloat16
    W_fc = np.asarray(W_fc, np.float32)
    W_proj = np.asarray(W_proj, np.float32)

    s_wfc = _scale_of(W_fc)
    s_wp = _scale_of(W_proj)
    qwfc = _quant(W_fc, s_wfc)
    qwp = _quant(W_proj, s_wp)

    # [k, ff] -> [mo, p(k%128), ko, c(ff%512)] -> chunks [2mo+half, p, 8, 512]
    wfc_dev = (
        qwfc.reshape(KO1, 128, MO1, 512)
        .transpose(2, 1, 0, 3)
        .astype(bf16)
        .reshape(MO1, 128, 2, W2CH, 512)
        .transpose(0, 2, 1, 3, 4)
        .reshape(2 * MO1, 128, W2CH, 512)
    )
    # [k, d] -> [no, p(k%128), ko, c(d%512)] -> chunks [8no+g, p, 8, 512]
    wpj_dev = (
        qwp.reshape(KO2, 128, NO2, 512)
        .transpose(2, 1, 0, 3)
        .astype(bf16)
        .reshape(NO2, 128, NCH2, W2CH, 512)
        .transpose(0, 2, 1, 3, 4)
        .reshape(NO2 * NCH2, 128, W2CH, 512)
    )
    wchunks = np.concatenate([wfc_dev, wpj_dev], axis=0)  # [64, 128, 8, 512]
    afc = np.asarray(A_fc, np.float32).reshape(KO1, 128, R).transpose(1, 0, 2)
    apj = np.asarray(A_proj, np.float32).reshape(KO2, 128, R).transpose(1, 0, 2)
    apk_dev = np.concatenate([afc, apj], axis=1).astype(bf16)  # [128, 80, 16]
    bpk_dev = np.concatenate(
        [np.asarray(B_fc, np.float32), np.asarray(B_proj, np.float32)], axis=1
    ).astype(bf16)  # [16, 10240]
    # chunks 68-71: A pack on chunk 68, B_fc on 69-70, B_proj on 71
    extra = np.zeros((4, 128, 4096), bf16)
    extra[0, :, 0:1280] = apk_dev.reshape(128, 1280)
    extra[1, 0:R, :] = bpk_dev[:, 0:4096]
    extra[2, 0:R, :] = bpk_dev[:, 4096:8192]
    extra[3, 0:R, 0:2048] = bpk_dev[:, 8192:10240]
    wchunks = np.ascontiguousarray(
        np.concatenate([wchunks, extra.reshape(4, 128, W2CH, 512)], axis=0)
    )  # [68, 128, 8, 512]
    fpk_dev = np.empty((128, M64 + D + 4), np.float32)
    fpk_dev[:, 0:M64] = np.asarray(b_fc, np.float32).reshape(M64, 128).T
    fpk_dev[:, M64 : M64 + D] = np.asarray(b_proj, np.float32)[None, :]
    return {
        "wchunks": wchunks,
        "fpk": fpk_dev,
        "s_wfc": s_wfc,
        "s_wp": s_wp,
    }


def _prep_inputs(hidden_states, W_fc, b_fc, A_fc, B_fc, W_proj, b_proj, A_proj, B_proj):
    bf16 = ml_dtypes.bfloat16
    w = _prep_weights(W_fc, b_fc, A_fc, B_fc, W_proj, b_proj, A_proj, B_proj)
    x = np.ascontiguousarray(np.asarray(hidden_states, np.float32).reshape(T, D))
    s_x = _scale_of(x)
    qx = _quant(x, s_x)
    s1 = np.float32(s_x * w["s_wfc"])
    c1 = np.float32(np.float32(2.0) / s1)
    fpk = w["fpk"].copy()
    fpk[:, M64 + D :] = np.array([s1, c1, w["s_wp"], 0.0], np.float32)[None, :]

    shared = {"fpk": fpk}
    in_maps = []
    for c in range(NCORES):
        xc = x[c * TC : (c + 1) * TC]  # [TC, D]
        qxc = qx[c * TC : (c + 1) * TC]
        qxT = qxc.T.reshape(KO1, 128, TC).transpose(1, 0, 2)
        xT = xc.T.reshape(KO1, 128, TC).transpose(1, 0, 2)
        # [128, 32ko, 512] -> 4 chunks [c, 128, 8, 512], then weight chunks
        xpk = (
            np.concatenate([qxT, xT], axis=1)
            .astype(bf16)
            .reshape(128, 4, W2CH, TC)
            .transpose(1, 0, 2, 3)
        )
        blob = np.ascontiguousarray(np.concatenate([xpk, w["wchunks"]], axis=0))
        in_maps.append({**shared, "blob": blob})
    return in_maps


def _get_runner(**build_kwargs):
    """Build the Bass program once and wrap it in a cached jitted shard_map
    executable (adapted from concourse.bass2jax.run_bass_via_pjrt)."""
    key = ("runner", tuple(sorted(build_kwargs.items())))
    if key in _CACHE:
        return _CACHE[key]
    nc = _build_nc(**build_kwargs)
    n_cores_ = build_kwargs.get("n_cores", NCORES)
    runner = _runner_from_nc(nc, n_cores_)
    _CACHE[key] = runner
    return runner


def _runner_from_nc(nc, n_cores_):
    import jax
    import jax.numpy as jnp  # noqa: F401
    from jax.experimental.shard_map import shard_map
    from jax.sharding import Mesh, PartitionSpec

    from concourse import bass2jax, mybir

    bass2jax.install_neuronx_cc_hook()
    assert nc.dbg_addr is None
    partition_name = nc.partition_id_tensor.name if nc.partition_id_tensor else None

    in_names = []
    out_names = []
    out_avals = []
    for alloc in nc.m.functions[0].allocations:
        if not isinstance(alloc, mybir.MemoryLocationSet):
            continue
        name = alloc.memorylocations[0].name
        if alloc.kind == "ExternalInput":
            if name != partition_name:
                in_names.append(name)
        elif alloc.kind == "ExternalOutput":
            out_names.append(name)
            out_avals.append(
                jax.core.ShapedArray(tuple(alloc.tensor_shape), mybir.dt.np(alloc.dtype))
            )
    all_in_names = tuple(in_names) + tuple(out_names)
    if partition_name is not None:
        all_in_names = all_in_names + (partition_name,)
    n_params = len(in_names)
    n_outs = len(out_names)

    def _body(*args):
        operands = list(args)
        if partition_name is not None:
            operands.append(bass2jax.partition_id_tensor())
        outs = bass2jax._bass_exec_p.bind(
            *operands,
            out_avals=tuple(out_avals),
            in_names=all_in_names,
            out_names=tuple(out_names),
            lowering_input_output_aliases=(),
            sim_require_finite=True,
            sim_require_nnan=True,
            nc=nc,
        )
        return tuple(outs)

    devices = jax.devices()[:n_cores_]
    assert len(devices) == n_cores_, f"need {n_cores_} devices, have {len(jax.devices())}"
    mesh = Mesh(np.asarray(devices), ("core",))
    # No donation: the kernel writes every element of its outputs, so the
    # zero "output" operands are unused placeholders (keep_unused) and one
    # staged set can be reused across calls -- donation would consume them.
    in_specs = (PartitionSpec("core"),) * (n_params + n_outs)
    out_specs = (PartitionSpec("core"),) * n_outs
    fn = jax.jit(
        shard_map(
            _body, mesh=mesh, in_specs=in_specs, out_specs=out_specs, check_rep=False
        ),
        keep_unused=True,
    )
    runner = {
        "fn": fn,
        "in_names": in_names,
        "out_names": out_names,
        "out_avals": out_avals,
        "mesh": mesh,
        "nc": nc,
    }
    runner["n_cores"] = n_cores_
    return runner


def _zero_outs(runner):
    n = runner["n_cores"]
    return [
        np.zeros((n * a.shape[0], *a.shape[1:]), a.dtype) for a in runner["out_avals"]
    ]


def _concat_inputs(in_maps, in_names):
    return [
        np.concatenate([m[name] for m in in_maps], axis=0) for name in in_names
    ]


def kernel(hidden_states, W_fc, b_fc, A_fc, B_fc, W_proj, b_proj, A_proj, B_proj):
    global LAST_RESULT
    runner = _get_runner()
    in_maps = _prep_inputs(
        hidden_states, W_fc, b_fc, A_fc, B_fc, W_proj, b_proj, A_proj, B_proj
    )
    concat_in = _concat_inputs(in_maps, runner["in_names"])
    out_arrs = runner["fn"](*concat_in, *_zero_outs(runner))
    (out_global,) = [np.asarray(a) for a in out_arrs]
    # out_global: [NCORES*MT, 128, D] -> per-core [MT,128,D] -> tokens x D
    out = out_global.reshape(T, D).astype(np.float32)
    return out.reshape(B_, S, D)


def _dummy_in_maps():
    rng = np.random.default_rng(0)
    dummy = {
        "hidden_states": rng.standard_normal((B_, S, D), dtype=np.float32),
        "W_fc": rng.standard_normal((D, DFF), dtype=np.float32) / 45.0,
        "b_fc": np.zeros(DFF, np.float32),
        "A_fc": rng.standard_normal((D, R), dtype=np.float32) / 45.0,
        "B_fc": rng.standard_normal((R, DFF), dtype=np.float32) * 0.01,
        "W_proj": rng.standard_normal((DFF, D), dtype=np.float32) / 90.0,
        "b_proj": np.zeros(D, np.float32),
        "A_proj": rng.standard_normal((DFF, R), dtype=np.float32) / 90.0,
        "B_proj": rng.standard_normal((R, D), dtype=np.float32) * 0.01,
    }
    return _prep_inputs(**dummy)


def bench(n_iters=20, in_maps=None, rounds=5, **build_kwargs):
    """Per-iteration wall time of the full kernel (one dispatch per forward
    pass, device-resident inputs), best of ``rounds`` runs of ``n_iters``
    iterations each.  The axon tunnel's dispatch cost degrades one-sidedly
    over a session, so min-of-rounds is the robust steady-state estimate."""
    import time

    import jax
    from jax.sharding import NamedSharding, PartitionSpec

    runner = _get_runner(**build_kwargs)
    if in_maps is None:
        in_maps = _dummy_in_maps()
    concat_in = _concat_inputs(in_maps, runner["in_names"])
    sharding = NamedSharding(runner["mesh"], PartitionSpec("core"))
    dev_in = [jax.device_put(a, sharding) for a in concat_in]
    for a in dev_in:
        jax.block_until_ready(a)

    zeros = [jax.device_put(z, sharding) for z in _zero_outs(runner)]
    for z in zeros:
        jax.block_until_ready(z)
    best = None
    for r in range(rounds):
        out = runner["fn"](*dev_in, *zeros)  # warmup (compile on r=0)
        jax.block_until_ready(out)
        t0 = time.time()
        for i in range(n_iters):
            out = runner["fn"](*dev_in, *zeros)
        jax.block_until_ready(out)
        dt = (time.time() - t0) / n_iters
        if best is None or dt < best:
            best = dt
    return best


def bench_device_loop(n_iters=20, in_maps=None, rounds=2, flags=()):
    """Device-side steady-state throughput: one dispatch executes a NEFF whose
    body is a hardware For loop running the forward pass ``n_iters`` times
    back-to-back, amortizing the per-call tunnel dispatch (~1.7ms on this
    setup).  The cross-core AllReduce desyncs the collective fabric when
    executed inside a hardware loop here, so this diagnostic runs the
    no_collective build (identical kernel minus the inter-core max exchange)."""
    import time

    import jax
    from jax.sharding import NamedSharding, PartitionSpec

    if "no_collective" not in flags:
        flags = ("no_collective",) + tuple(flags)
    runner = _get_runner(loop_k=n_iters, flags=flags)
    if in_maps is None:
        in_maps = _dummy_in_maps()
    concat_in = _concat_inputs(in_maps, runner["in_names"])
    sharding = NamedSharding(runner["mesh"], PartitionSpec("core"))
    dev_in = [jax.device_put(a, sharding) for a in concat_in]
    for a in dev_in:
        jax.block_until_ready(a)

    best = None
    for r in range(rounds + 1):  # first call = warmup (compile), not timed
        zs = [jax.device_put(z, sharding) for z in _zero_outs(runner)]
        for z in zs:
            jax.block_until_ready(z)
        t0 = time.time()
        out = runner["fn"](*dev_in, *zs)
        jax.block_until_ready(out)
        dt = (time.time() - t0) / n_iters
        if r > 0 and (best is None or dt < best):
            best = dt
    return best
